# revision 22
# baseline (speedup 1.0000x reference)
"""Trainium2 Bass kernel for nn_HA_unit (gnn_message_passing).

Math (per batch b, N = H*W spatial positions):
  wfeat = BN1(w1 @ x)                       [IC, N]   (BN folded on host)
  iw    = wfeat^T wfeat * IC^-0.5           [N, N]    symmetric
  nodes = node_w @ x + node_b               [N, IC]
  b0    = (sigmoid(iw) >= delta)            [N, N]    binary, symmetric
  bh_k  = b0^k  (k = 1, 2, 3)               exact integer counts
  hop_k = hopw_k @ (softmax(bh_k o iw) @ nodes)^T + hopb_k
  xp    = BNf(fuse_w @ concat(hops))
  out   = BNr(res_w @ concat(x[:IC], xp))

Sharding: 8 cores = 4 batches x 2 halves of N. Core (b, h) receives x[b]
with spatial positions rolled by h*N/2 so that its rows are always 0..N/2-1
(identical SPMD program, data-only difference). Each core computes the full
symmetric b0 locally (no collectives); the b0^2 / b0^3 matmuls are sharded
by output rows.

Perf notes vs the first working version:
  - all dense matmuls run on f16 operands (1 cyc/row) instead of f32
    (4 cyc/row); fp16 keeps ~3 decimal digits which is far inside the
    2e-2 gate (measured ~2e-4 end to end).
  - bh2 = b0 @ b0 uses fp8 DoubleRow (0.5 cyc/row).  b0 is exactly
    representable in fp8; bh2 (counts <= 2048) is exact in f16.
  - softmax hop iterations are interleaved with the big matmul phases
    (hop0 with C, hop1 with D, hop2 with F) so vector/scalar work hides
    under TensorE.
  - softmax 1/Z normalisation is folded into the [128,256] t eviction
    instead of a [128,4096] scaled copy.
  - s @ nodes runs with sT tiles as the stationary operand and free
    dim 256, halving PE time vs free-dim-128 with nodes stationary.
  - nodes / hops stay SBUF-resident (no DRAM round trips).
"""

import sys

sys.path.insert(0, "/opt/trn_rl_repo")

import numpy as np

P = 128


def _build(cin, ic, n, r, hop, thr):
    from concourse import bass, tile, bacc
    import concourse.mybir as mybir
    from concourse.masks import make_identity

    f32 = mybir.dt.float32
    f16 = mybir.dt.float16
    fp8 = mybir.dt.float8e4
    AF = mybir.ActivationFunctionType
    ALU = mybir.AluOpType
    AX = mybir.AxisListType
    DR = mybir.MatmulPerfMode.DoubleRow

    ncin = cin // P          # K-chunks over input channels
    nic = ic // P            # chunks over inter channels
    nkn = n // P             # K-chunks over N
    nrt = r // P             # our row tiles
    FB = min(512, n)         # free-dim blocking
    nfb = n // FB
    hc = hop * ic
    nhc = hc // P
    cout = cin
    ncout = cout // P
    nxc = ic // P            # x residual slice chunks (x[:ic])

    # bias_pack columns: [b1(nic) | hop(hop*nic) | fuse(nic) | res(ncout)]
    C_B1 = 0
    C_HOP = C_B1 + nic
    C_FUSE = C_HOP + hop * nic
    C_RES = C_FUSE + nic
    NBIAS = C_RES + ncout

    nc = bacc.Bacc("TRN2", target_bir_lowering=False, debug=False)

    xb = nc.dram_tensor("xb", [cin, n], f16, kind="ExternalInput")
    w1T = nc.dram_tensor("w1T", [cin, ic], f16, kind="ExternalInput")
    nodeT = nc.dram_tensor("nodeT", [cin, ic], f16, kind="ExternalInput")
    nbrow = nc.dram_tensor("nbrow", [1, ic], f16, kind="ExternalInput")
    hopT = nc.dram_tensor("hopT", [hop, ic, ic], f16, kind="ExternalInput")
    fuseT = nc.dram_tensor("fuseT", [hc, ic], f16, kind="ExternalInput")
    resT = nc.dram_tensor("resT", [2 * ic, cout], f16, kind="ExternalInput")
    biases = nc.dram_tensor("biases", [P, NBIAS], f32, kind="ExternalInput")
    out = nc.dram_tensor("out", [cout, r], f32, kind="ExternalOutput")

    with tile.TileContext(nc) as tc:
        with (
            tc.tile_pool(name="dram", bufs=1, space="DRAM") as dpool,
            tc.tile_pool(name="consts", bufs=1) as consts,
            tc.tile_pool(name="res", bufs=1) as resid,
            tc.tile_pool(name="psMM", bufs=2, space="PSUM") as psMM,
            tc.tile_pool(name="psTR", bufs=2, space="PSUM") as psTR,
            tc.tile_pool(name="psTC", bufs=2, space="PSUM") as psTC,
            tc.tile_pool(name="psSM", bufs=2, space="PSUM") as psSM,
        ):
            b0q = dpool.tile([n, n], fp8, tag="b0q")
            iwq = dpool.tile([r, n], f32, tag="iwq")
            bh2 = dpool.tile([r, n], f32, tag="bh2")
            bh2T = dpool.tile([n, r], f16, tag="bh2T")
            bh3 = dpool.tile([r, n], f32, tag="bh3")

            identh = consts.tile([P, P], f16, tag="identh")
            make_identity(nc, identh[:])
            bias_sb = consts.tile([P, NBIAS], f32, tag="bias_sb")
            nc.sync.dma_start(bias_sb[:], biases[:])
            ones1 = consts.tile([1, P], f16, tag="ones1")
            nc.vector.memset(ones1[:], 1.0)
            nbrow_sb = consts.tile([1, ic], f16, tag="nbrow_sb")
            nc.sync.dma_start(nbrow_sb[:], nbrow[:])

            # persistent SBUF residents
            nodes_sb = resid.tile([P, nkn, ic], f16, tag="nodes_sb")
            hops_sb = resid.tile([P, hop * nic, r], f16, tag="hops_sb")
            hopT_sb = resid.tile([P, hop, nic, ic], f16, tag="hopT_sb")
            fuseT_sb = resid.tile([P, nhc, ic], f16, tag="fuseT_sb")
            resT_sb = resid.tile([P, 2 * nic, cout], f16, tag="resT_sb")
            xres_sb = resid.tile([P, nxc, r], f16, tag="xres_sb")
            for i in range(hop):
                nc.sync.dma_start(
                    hopT_sb[:, i, :, :],
                    hopT[i, :, :].rearrange("(k p) o -> p k o", p=P),
                )
            nc.sync.dma_start(
                fuseT_sb[:], fuseT[:, :].rearrange("(k p) o -> p k o", p=P)
            )
            nc.sync.dma_start(
                resT_sb[:], resT[:, :].rearrange("(k p) o -> p k o", p=P)
            )
            nc.sync.dma_start(
                xres_sb[:], xb[0:ic, 0:r].rearrange("(k p) q -> p k q", p=P)
            )

            # one softmax-hop row-tile (rt): bht==None means hop0 (mask
            # recomputed from iw on the fly)
            def hop_iter(pool, i, rt, bht):
                iwt = pool.tile([P, n], f32, tag="E_iw")
                nc.sync.dma_start(iwt[:], iwq[rt * P:(rt + 1) * P, :])
                # logits in-place into iwt (baseline-proven dtype combos:
                # f32*fp8 on DVE for hop0, f32*f32 on POOL for hops 1-2)
                if i == 0:
                    nc.vector.tensor_mul(iwt[:], iwt[:], bht[:])
                else:
                    nc.gpsimd.tensor_mul(iwt[:], iwt[:], bht[:])
                nmax = pool.tile([P, 1], f32, tag="E_nm")
                nc.vector.tensor_reduce(
                    nmax[:], iwt[:], axis=AX.X, op=ALU.max, negate=True
                )
                zt = pool.tile([P, 1], f32, tag="E_z")
                pt_ = pool.tile([P, n], f16, tag="E_p")
                nc.scalar.activation(
                    pt_[:], iwt[:], AF.Exp, bias=nmax[:], accum_out=zt[:]
                )
                rz = pool.tile([P, 1], f32, tag="E_rz")
                nc.vector.reciprocal(rz[:], zt[:])
                sT = pool.tile([P, nfb, FB], f16, tag="E_sT", bufs=1)
                for jb in range(nfb):
                    pst = psTR.tile([P, FB], f16, tag="trp")
                    for q in range(FB // P):
                        nc.tensor.transpose(
                            pst[:, q * P:(q + 1) * P],
                            pt_[:, jb * FB + q * P:jb * FB + (q + 1) * P],
                            identh[:],
                        )
                    nc.vector.tensor_copy(sT[:, jb, :], pst[:])
                tps = psTC.tile([P, ic], f32, tag="E_tps")
                for j in range(nkn):
                    nc.tensor.matmul(
                        tps[:],
                        sT[:, j // (FB // P), (j % (FB // P)) * P:
                           (j % (FB // P)) * P + P],
                        nodes_sb[:, j, :],
                        start=(j == 0),
                        stop=(j == nkn - 1),
                    )
                t_sb = pool.tile([P, ic], f16, tag="E_t")
                nc.vector.tensor_scalar_mul(t_sb[:], tps[:], rz[:])
                tT = pool.tile([P, nic, P], f16, tag="E_tT")
                for q in range(nic):
                    ptq = psSM.tile([P, P], f16, tag="E_sm")
                    nc.tensor.transpose(
                        ptq[:], t_sb[:, q * P:(q + 1) * P], identh[:]
                    )
                    nc.vector.tensor_copy(tT[:, q, :], ptq[:])
                for o in range(nic):
                    ph = psSM.tile([P, P], f32, tag="E_sm")
                    for c in range(nic):
                        nc.tensor.matmul(
                            ph[:],
                            hopT_sb[:, i, c, o * P:(o + 1) * P],
                            tT[:, c, :],
                            start=(c == 0),
                            stop=(c == nic - 1),
                        )
                    nc.vector.tensor_scalar_add(
                        hops_sb[:, i * nic + o, rt * P:(rt + 1) * P],
                        ph[:],
                        bias_sb[
                            :, C_HOP + i * nic + o:C_HOP + i * nic + o + 1
                        ],
                    )

            # ---------------- Phase A: wfeat + nodes ----------------
            with (
                tc.tile_pool(name="pa", bufs=1) as pa,
                tc.tile_pool(name="evA", bufs=2) as evict,
            ):
                x_sb = pa.tile([P, ncin, n], f16, tag="x_sb")
                nc.sync.dma_start(
                    x_sb[:], xb[:, :].rearrange("(k p) n -> p k n", p=P)
                )
                w1T_sb = pa.tile([P, ncin, ic], f16, tag="w1T_sb")
                nc.sync.dma_start(
                    w1T_sb[:], w1T[:, :].rearrange("(k p) o -> p k o", p=P)
                )
                nodeT_sb = pa.tile([P, ncin, ic], f16, tag="nodeT_sb")
                nc.sync.dma_start(
                    nodeT_sb[:], nodeT[:, :].rearrange("(k p) o -> p k o", p=P)
                )
                wf_sb = pa.tile([P, nic, n], f16, tag="wf_sb")

                for oc in range(nic):
                    for f in range(nfb):
                        ps = psMM.tile([P, FB], f32, tag="mm")
                        for k in range(ncin):
                            nc.tensor.matmul(
                                ps[:],
                                w1T_sb[:, k, oc * P:(oc + 1) * P],
                                x_sb[:, k, f * FB:(f + 1) * FB],
                                start=(k == 0),
                                stop=(k == ncin - 1),
                            )
                        nc.vector.tensor_scalar_add(
                            wf_sb[:, oc, f * FB:(f + 1) * FB],
                            ps[:],
                            bias_sb[:, C_B1 + oc:C_B1 + oc + 1],
                        )

                for nt in range(nkn):
                    ps = psTC.tile([P, ic], f32, tag="E_tps")
                    for k in range(ncin):
                        nc.tensor.matmul(
                            ps[:],
                            x_sb[:, k, nt * P:(nt + 1) * P],
                            nodeT_sb[:, k, :],
                            start=(k == 0),
                            stop=False,
                        )
                    nc.tensor.matmul(
                        ps[:], ones1[:], nbrow_sb[:], start=False, stop=True
                    )
                    nc.vector.tensor_copy(nodes_sb[:, nt, :], ps[:])

                # ---------------- Phase B: iw + b0 ----------------
                for pc in range(nkn):
                    for f in range(nfb):
                        ps = psMM.tile([P, FB], f32, tag="mm")
                        for k in range(nic):
                            nc.tensor.matmul(
                                ps[:],
                                wf_sb[:, k, pc * P:(pc + 1) * P],
                                wf_sb[:, k, f * FB:(f + 1) * FB],
                                start=(k == 0),
                                stop=(k == nic - 1),
                            )
                        b0t = evict.tile([P, FB], fp8, tag="b0t")
                        nc.vector.tensor_scalar(
                            b0t[:], ps[:], thr, None, op0=ALU.is_ge
                        )
                        nc.sync.dma_start(
                            b0q[pc * P:(pc + 1) * P, f * FB:(f + 1) * FB], b0t[:]
                        )
                        if pc * P < r:
                            iwt = evict.tile([P, FB], f32, tag="iwt")
                            nc.scalar.activation(iwt[:], ps[:], AF.Copy)
                            nc.sync.dma_start(
                                iwq[pc * P:(pc + 1) * P, f * FB:(f + 1) * FB],
                                iwt[:],
                            )

            # ------- Phase C: bh2 = b0 @ b0 (fp8 DoubleRow, exact) -------
            # + hop0 softmax row-tiles interleaved
            with (
                tc.tile_pool(name="pc_lhs", bufs=1) as pcl,
                tc.tile_pool(name="pc_rhs", bufs=2) as pcr,
                tc.tile_pool(name="evCp", bufs=2) as evict,
                tc.tile_pool(name="pe0", bufs=2) as pe0,
            ):
                RQH = nrt // 2
                for rqh in range(2):
                    lh = pcl.tile([P, nkn, RQH * P], fp8, tag="lhC")
                    nc.sync.dma_start(
                        lh[:],
                        b0q[:, rqh * RQH * P:(rqh + 1) * RQH * P].rearrange(
                            "(k p) q -> p k q", p=P
                        ),
                    )
                    for mc in range(nfb):
                        rt_ = pcr.tile([P, nkn, FB], fp8, tag="rhsC")
                        nc.sync.dma_start(
                            rt_[:],
                            b0q[:, mc * FB:(mc + 1) * FB].rearrange(
                                "(k p) q -> p k q", p=P
                            ),
                        )
                        for rq in range(RQH):
                            ps = psMM.tile([P, FB], f32, tag="mm")
                            for k in range(0, nkn, 2):
                                nc.tensor.matmul(
                                    ps[:],
                                    lh[:, k:k + 2, rq * P:(rq + 1) * P],
                                    rt_[:, k:k + 2, :],
                                    start=(k == 0),
                                    stop=(k == nkn - 2),
                                    perf_mode=DR,
                                )
                            rg = (rqh * RQH + rq) * P
                            ev32 = evict.tile([P, FB], f32, tag="evC32")
                            nc.vector.tensor_copy(ev32[:], ps[:])
                            nc.sync.dma_start(
                                bh2[rg:rg + P, mc * FB:(mc + 1) * FB], ev32[:]
                            )
                            ev = evict.tile([P, FB], f16, tag="evC")
                            nc.vector.tensor_copy(ev[:], ps[:])
                            pst = psTR.tile([P, FB], f16, tag="trp")
                            for q in range(FB // P):
                                nc.tensor.transpose(
                                    pst[:, q * P:(q + 1) * P],
                                    ev[:, q * P:(q + 1) * P],
                                    identh[:],
                                )
                            tT_ = evict.tile([P, FB // P, P], f16, tag="tTC")
                            for q in range(FB // P):
                                nc.vector.tensor_copy(
                                    tT_[:, q, :], pst[:, q * P:(q + 1) * P]
                                )
                            nc.sync.dma_start(
                                bh2T[
                                    mc * FB:(mc + 1) * FB, rg:rg + P
                                ].rearrange("(j p) q -> p j q", p=P),
                                tT_[:],
                            )
                        rt0 = rqh * nfb + mc
                        if rt0 < nrt:
                            bht = pe0.tile([P, n], fp8, tag="E_bh0")
                            nc.sync.dma_start(
                                bht[:], b0q[rt0 * P:(rt0 + 1) * P, :]
                            )
                            hop_iter(pe0, 0, rt0, bht)

            # ------- Phase D: bh3 = bh2 @ b0 (f16 x fp8, exact f32) -------
            # + hop1 softmax row-tiles interleaved
            RH = min(512, r)
            nrh = r // RH
            nrq = RH // P
            with (
                tc.tile_pool(name="pd_lhs", bufs=1) as pdl,
                tc.tile_pool(name="pd_rhs", bufs=2) as pdr,
                tc.tile_pool(name="evDp", bufs=2) as evict,
                tc.tile_pool(name="pe1", bufs=2) as pe1,
            ):
                for rh in range(nrh):
                    lh = pdl.tile([P, nkn, RH], f16, tag="lhD")
                    nc.sync.dma_start(
                        lh[:],
                        bh2T[:, rh * RH:(rh + 1) * RH].rearrange(
                            "(k p) q -> p k q", p=P
                        ),
                    )
                    for mc in range(nfb):
                        rt_ = pdr.tile([P, nkn, FB], fp8, tag="rhsD")
                        nc.sync.dma_start(
                            rt_[:],
                            b0q[:, mc * FB:(mc + 1) * FB].rearrange(
                                "(k p) q -> p k q", p=P
                            ),
                        )
                        for rq in range(nrq):
                            ps = psMM.tile([P, FB], f32, tag="mm")
                            for k in range(nkn):
                                nc.tensor.matmul(
                                    ps[:],
                                    lh[:, k, rq * P:(rq + 1) * P],
                                    rt_[:, k, :],
                                    start=(k == 0),
                                    stop=(k == nkn - 1),
                                )
                            ev = evict.tile([P, FB], f32, tag="evD")
                            nc.vector.tensor_copy(ev[:], ps[:])
                            rg = rh * RH + rq * P
                            nc.sync.dma_start(
                                bh3[rg:rg + P, mc * FB:(mc + 1) * FB], ev[:]
                            )
                    for rt in range(rh * nrq, (rh + 1) * nrq):
                        bht = pe1.tile([P, n], f32, tag="E_bh1", bufs=1)
                        nc.sync.dma_start(bht[:], bh2[rt * P:(rt + 1) * P, :])
                        hop_iter(pe1, 1, rt, bht)

            # ------- Phase E2 + F: hop2 softmax, fuse, residual -------
            FR = min(512, r)
            nrf = r // FR
            with (
                tc.tile_pool(name="pf", bufs=1) as pf,
                tc.tile_pool(name="evFp", bufs=2) as evict,
                tc.tile_pool(name="pe2", bufs=2) as pe2,
            ):
                xp_sb = pf.tile([P, nic, r], f16, tag="xp_sb")
                for rf in range(nrf):
                    for rt in range(rf * (nrt // nrf), (rf + 1) * (nrt // nrf)):
                        bht = pe2.tile([P, n], f32, tag="E_bh2", bufs=1)
                        nc.sync.dma_start(bht[:], bh3[rt * P:(rt + 1) * P, :])
                        hop_iter(pe2, 2, rt, bht)
                    for o in range(nic):
                        ps = psMM.tile([P, FR], f32, tag="mm")
                        for k in range(nhc):
                            nc.tensor.matmul(
                                ps[:],
                                fuseT_sb[:, k, o * P:(o + 1) * P],
                                hops_sb[:, k, rf * FR:(rf + 1) * FR],
                                start=(k == 0),
                                stop=(k == nhc - 1),
                            )
                        nc.vector.tensor_scalar_add(
                            xp_sb[:, o, rf * FR:(rf + 1) * FR],
                            ps[:],
                            bias_sb[:, C_FUSE + o:C_FUSE + o + 1],
                        )

                for o in range(ncout):
                    for rf in range(nrf):
                        ps = psMM.tile([P, FR], f32, tag="mm")
                        for k in range(nxc):
                            nc.tensor.matmul(
                                ps[:],
                                resT_sb[:, k, o * P:(o + 1) * P],
                                xres_sb[:, k, rf * FR:(rf + 1) * FR],
                                start=(k == 0),
                                stop=False,
                            )
                        for k in range(nic):
                            nc.tensor.matmul(
                                ps[:],
                                resT_sb[:, nxc + k, o * P:(o + 1) * P],
                                xp_sb[:, k, rf * FR:(rf + 1) * FR],
                                start=False,
                                stop=(k == nic - 1),
                            )
                        ev = evict.tile([P, FR], f32, tag="evF")
                        nc.vector.tensor_scalar_add(
                            ev[:], ps[:], bias_sb[:, C_RES + o:C_RES + o + 1]
                        )
                        nc.sync.dma_start(
                            out[o * P:(o + 1) * P, rf * FR:(rf + 1) * FR], ev[:]
                        )

    nc.compile()
    return nc


def _host_prep(inputs, cin, ic, n, r, hop, eps):
    """Fold BN into weights; build per-core input maps."""

    def f32(a):
        return np.ascontiguousarray(np.asarray(a, dtype=np.float32))

    x = f32(inputs["x"])
    B = x.shape[0]
    xf = x.reshape(B, cin, n)

    s4 = float(ic) ** -0.25
    inv1 = 1.0 / np.sqrt(f32(inputs["bn1_v"]) + eps) * f32(inputs["bn1_g"])
    w1_eff = (inv1[:, None] * f32(inputs["w1_w"])) * s4
    b1_eff = (f32(inputs["w1_b"]) * inv1 + f32(inputs["bn1_b"])
              - f32(inputs["bn1_m"]) * inv1) * s4

    invf = 1.0 / np.sqrt(f32(inputs["bnf_v"]) + eps) * f32(inputs["bnf_g"])
    fuse_eff = invf[:, None] * f32(inputs["fuse_w"])
    fuse_b_eff = (f32(inputs["fuse_b"]) * invf + f32(inputs["bnf_b"])
                  - f32(inputs["bnf_m"]) * invf)

    invr = 1.0 / np.sqrt(f32(inputs["bnr_v"]) + eps) * f32(inputs["bnr_g"])
    res_eff = invr[:, None] * f32(inputs["res_w"])
    res_b_eff = (f32(inputs["res_b"]) * invr + f32(inputs["bnr_b"])
                 - f32(inputs["bnr_m"]) * invr)

    delta = float(np.asarray(inputs["delta"]).reshape(-1)[0])
    if delta <= 0.0:
        thr = -3.0e38
    elif delta >= 1.0:
        thr = 3.0e38
    else:
        thr = float(np.log(delta / (1.0 - delta)))

    nic = ic // P
    ncout = cin // P
    nbias = nic + hop * nic + nic + ncout
    bias_pack = np.zeros((P, nbias), np.float32)
    col = 0
    for oc in range(nic):
        bias_pack[:, col] = b1_eff[oc * P:(oc + 1) * P]
        col += 1
    hop_b = f32(inputs["hop_b"])
    for i in range(hop):
        for oc in range(nic):
            bias_pack[:, col] = hop_b[i, oc * P:(oc + 1) * P]
            col += 1
    for oc in range(nic):
        bias_pack[:, col] = fuse_b_eff[oc * P:(oc + 1) * P]
        col += 1
    for oc in range(ncout):
        bias_pack[:, col] = res_b_eff[oc * P:(oc + 1) * P]
        col += 1

    f16 = np.float16
    shared = {
        "w1T": np.ascontiguousarray(w1_eff.T.astype(f16)),
        "nodeT": np.ascontiguousarray(f32(inputs["node_w"]).T.astype(f16)),
        "nbrow": f32(inputs["node_b"]).reshape(1, ic).astype(f16),
        "hopT": np.ascontiguousarray(
            f32(inputs["hop_w"]).transpose(0, 2, 1).astype(f16)
        ),
        "fuseT": np.ascontiguousarray(fuse_eff.T.astype(f16)),
        "resT": np.ascontiguousarray(res_eff.T.astype(f16)),
        "biases": bias_pack,
    }

    n_cores = (B * n) // r
    halves = n // r
    in_maps = []
    for c in range(n_cores):
        b, h = c // halves, c % halves
        perm = (np.arange(n) + h * r) % n
        m = dict(shared)
        m["xb"] = np.ascontiguousarray(xf[b][:, perm].astype(f16))
        in_maps.append(m)
    return in_maps, thr


_BUILD_CACHE = {}


def kernel(**inputs):
    from concourse import bass_utils

    cin, ic, hop, eps = 512, 256, 3, 1e-5
    x = np.asarray(inputs["x"])
    B, _, H, W = x.shape
    n = H * W
    n_cores = 8
    r = (B * n) // n_cores
    halves = n // r

    in_maps, thr = _host_prep(inputs, cin, ic, n, r, hop, eps)

    key = (cin, ic, n, r, hop, thr)
    if key not in _BUILD_CACHE:
        _BUILD_CACHE[key] = _build(cin, ic, n, r, hop, thr)
    nc = _BUILD_CACHE[key]

    res = bass_utils.run_bass_kernel_spmd(nc, in_maps, core_ids=list(range(n_cores)))

    out = np.empty((B, cin, n), np.float32)
    for c in range(n_cores):
        b, h = c // halves, c % halves
        out[b][:, h * r:(h + 1) * r] = res.results[c]["out"]
    return out.reshape(B, cin, H, W).astype(x.dtype)


# revision 25
# speedup vs baseline: 1.0469x; 1.0469x over previous
"""Trainium2 Bass kernel for nn_HA_unit (gnn_message_passing).

Math (per batch b, N = H*W spatial positions):
  wfeat = BN1(w1 @ x)                       [IC, N]   (BN folded on host)
  iw    = wfeat^T wfeat * IC^-0.5           [N, N]    symmetric
  nodes = node_w @ x + node_b               [N, IC]
  b0    = (sigmoid(iw) >= delta)            [N, N]    binary, symmetric
  bh_k  = b0^k  (k = 1, 2, 3)               exact integer counts
  hop_k = hopw_k @ (softmax(bh_k o iw) @ nodes)^T + hopb_k
  xp    = BNf(fuse_w @ concat(hops))
  out   = BNr(res_w @ concat(x[:IC], xp))

Sharding: 8 cores = 4 batches x 2 halves of N. Core (b, h) receives x[b]
with spatial positions rolled by h*N/2 so that its rows are always 0..N/2-1
(identical SPMD program, data-only difference). Each core computes the full
symmetric b0 locally (no collectives); the b0^2 / b0^3 matmuls are sharded
by output rows.

Performance structure:
  - all dense matmuls on f16 operands (1 cyc/row); fp8 DoubleRow
    (0.5 cyc/row) for b0 @ b0.  b0 is exact in fp8, bh2 (< 2048) exact
    in f16, bh3 exact in f32 PSUM.
  - single 4-deep PSUM pool for every accumulation group keeps TensorE
    several groups ahead of the evictions (HAM stays un-throttled).
  - softmax hop row-tiles run split-stage (stage1 = DMA+logits+exp on
    DVE/POOL/ACT, stage2 = transposes + matmuls on PE) with a one-block
    skew, interleaved into the big matmul phases: hop0 inside C, hop1
    and hop2 inside D.  Only fuse+residual remain as a tail.
  - nodes / hops stay SBUF-resident; evictions ride DVE, not ACT.
"""

import sys

sys.path.insert(0, "/opt/trn_rl_repo")

import numpy as np

P = 128


def _build(cin, ic, n, r, hop, thr):
    from concourse import bass, tile, bacc
    import concourse.mybir as mybir
    from concourse.masks import make_identity

    f32 = mybir.dt.float32
    f16 = mybir.dt.float16
    fp8 = mybir.dt.float8e4
    AF = mybir.ActivationFunctionType
    ALU = mybir.AluOpType
    AX = mybir.AxisListType
    DR = mybir.MatmulPerfMode.DoubleRow

    ncin = cin // P          # K-chunks over input channels
    nic = ic // P            # chunks over inter channels
    nkn = n // P             # K-chunks over N
    nrt = r // P             # our row tiles
    FB = min(512, n)         # free-dim blocking
    nfb = n // FB
    hc = hop * ic
    nhc = hc // P
    cout = cin
    ncout = cout // P
    nxc = ic // P            # x residual slice chunks (x[:ic])

    # bias_pack columns: [b1(nic) | nodeb(nic) | hop(hop*nic) | fuse(nic) | res(ncout)]
    C_B1 = 0
    C_NB = C_B1 + nic
    C_HOP = C_NB + nic
    C_FUSE = C_HOP + hop * nic
    C_RES = C_FUSE + nic
    NBIAS = C_RES + ncout

    nc = bacc.Bacc("TRN2", target_bir_lowering=False, debug=False)

    xb = nc.dram_tensor("xb", [cin, n], f16, kind="ExternalInput")
    w1T = nc.dram_tensor("w1T", [cin, ic], f16, kind="ExternalInput")
    nodeT = nc.dram_tensor("nodeT", [cin, ic], f16, kind="ExternalInput")
    hopT = nc.dram_tensor("hopT", [hop, ic, ic], f16, kind="ExternalInput")
    fuseT = nc.dram_tensor("fuseT", [hc, ic], f16, kind="ExternalInput")
    resT = nc.dram_tensor("resT", [2 * ic, cout], f16, kind="ExternalInput")
    biases = nc.dram_tensor("biases", [P, NBIAS], f32, kind="ExternalInput")
    out = nc.dram_tensor("out", [cout, r], f32, kind="ExternalOutput")

    with tile.TileContext(nc) as tc:
        with (
            tc.tile_pool(name="dram", bufs=1, space="DRAM") as dpool,
            tc.tile_pool(name="consts", bufs=1) as consts,
            tc.tile_pool(name="res", bufs=1) as resid,
            tc.tile_pool(name="psMM", bufs=4, space="PSUM") as psMM,
            tc.tile_pool(name="psTR", bufs=3, space="PSUM") as psTR,
        ):
            b0q = dpool.tile([n, n], fp8, tag="b0q")
            iwq = dpool.tile([r, n], f32, tag="iwq")
            bh2 = dpool.tile([r, n], f32, tag="bh2")
            bh2T = dpool.tile([n, r], f16, tag="bh2T")
            bh3 = dpool.tile([r, n], f32, tag="bh3")

            identh = consts.tile([P, P], f16, tag="identh")
            make_identity(nc, identh[:])
            bias_sb = consts.tile([P, NBIAS], f32, tag="bias_sb")
            nc.sync.dma_start(bias_sb[:], biases[:])

            # persistent SBUF residents
            nodes_sb = resid.tile([P, nkn, ic], f16, tag="nodes_sb")
            hops_sb = resid.tile([P, hop * nic, r], f16, tag="hops_sb")
            hopT_sb = resid.tile([P, hop, nic, ic], f16, tag="hopT_sb")
            for i in range(hop):
                nc.sync.dma_start(
                    hopT_sb[:, i, :, :],
                    hopT[i, :, :].rearrange("(k p) o -> p k o", p=P),
                )

            # ---- softmax hop row-tile, split in two stages ----
            # stage1: load iw row + bh row, logits, rowmax, exp (+rowsum)
            # stage2: transpose s, t = s @ nodes (scaled 1/Z), hop conv
            def hop_stage1(pool, i, rt):
                iwt = pool.tile([P, n], f32, tag="E_iw")
                nc.sync.dma_start(iwt[:], iwq[rt * P:(rt + 1) * P, :])
                if i == 0:
                    bht = pool.tile([P, n], fp8, tag="E_bh0")
                    nc.sync.dma_start(bht[:], b0q[rt * P:(rt + 1) * P, :])
                    nc.vector.tensor_mul(iwt[:], iwt[:], bht[:])
                else:
                    src = bh2 if i == 1 else bh3
                    bht = pool.tile([P, n], f32, tag="E_bh", bufs=2)
                    nc.sync.dma_start(bht[:], src[rt * P:(rt + 1) * P, :])
                    nc.gpsimd.tensor_mul(iwt[:], iwt[:], bht[:])
                nmax = pool.tile([P, 1], f32, tag="E_nm")
                nc.vector.tensor_reduce(
                    nmax[:], iwt[:], axis=AX.X, op=ALU.max, negate=True
                )
                zt = pool.tile([P, 1], f32, tag="E_z")
                pt_ = pool.tile([P, n], f16, tag="E_p")
                nc.scalar.activation(
                    pt_[:], iwt[:], AF.Exp, bias=nmax[:], accum_out=zt[:]
                )
                rz = pool.tile([P, 1], f32, tag="E_rz")
                nc.vector.reciprocal(rz[:], zt[:])
                return (pool, i, rt, pt_, rz)

            def hop_stage2(state):
                pool, i, rt, pt_, rz = state
                sT = pool.tile([P, nfb, FB], f16, tag="E_sT", bufs=1)
                for jb in range(nfb):
                    pst = psTR.tile([P, FB], f16, tag="trp")
                    for q in range(FB // P):
                        nc.tensor.transpose(
                            pst[:, q * P:(q + 1) * P],
                            pt_[:, jb * FB + q * P:jb * FB + (q + 1) * P],
                            identh[:],
                        )
                    nc.vector.tensor_copy(sT[:, jb, :], pst[:])
                tps = psMM.tile([P, FB], f32, tag="mm")
                for j in range(nkn):
                    nc.tensor.matmul(
                        tps[:, 0:ic],
                        sT[:, j // (FB // P), (j % (FB // P)) * P:
                           (j % (FB // P)) * P + P],
                        nodes_sb[:, j, :],
                        start=(j == 0),
                        stop=(j == nkn - 1),
                    )
                t_sb = pool.tile([P, ic], f16, tag="E_t")
                nc.vector.tensor_scalar_mul(t_sb[:], tps[:, 0:ic], rz[:])
                tT = pool.tile([P, nic, P], f16, tag="E_tT")
                for q in range(nic):
                    ptq = psTR.tile([P, P], f16, tag="trp")
                    nc.tensor.transpose(
                        ptq[:], t_sb[:, q * P:(q + 1) * P], identh[:]
                    )
                    nc.vector.tensor_copy(tT[:, q, :], ptq[:])
                for o in range(nic):
                    ph = psTR.tile([P, P], f32, tag="trp")
                    for c in range(nic):
                        nc.tensor.matmul(
                            ph[:],
                            hopT_sb[:, i, c, o * P:(o + 1) * P],
                            tT[:, c, :],
                            start=(c == 0),
                            stop=(c == nic - 1),
                        )
                    nc.vector.tensor_scalar_add(
                        hops_sb[:, i * nic + o, rt * P:(rt + 1) * P],
                        ph[:],
                        bias_sb[
                            :, C_HOP + i * nic + o:C_HOP + i * nic + o + 1
                        ],
                    )

            # ---------------- Phase A: wfeat + nodes ----------------
            with (
                tc.tile_pool(name="pa", bufs=1) as pa,
                tc.tile_pool(name="evA", bufs=2) as evict,
            ):
                x_sb = pa.tile([P, ncin, n], f16, tag="x_sb")
                nc.sync.dma_start(
                    x_sb[:], xb[:, :].rearrange("(k p) n -> p k n", p=P)
                )
                w1T_sb = pa.tile([P, ncin, ic], f16, tag="w1T_sb")
                nc.sync.dma_start(
                    w1T_sb[:], w1T[:, :].rearrange("(k p) o -> p k o", p=P)
                )
                nodeT_sb = pa.tile([P, ncin, ic], f16, tag="nodeT_sb")
                nc.sync.dma_start(
                    nodeT_sb[:], nodeT[:, :].rearrange("(k p) o -> p k o", p=P)
                )
                wf_sb = pa.tile([P, nic, n], f16, tag="wf_sb")
                ndT_sb = pa.tile([P, nic, n], f16, tag="ndT_sb")

                for oc in range(nic):
                    for f in range(nfb):
                        ps = psMM.tile([P, FB], f32, tag="mm")
                        for k in range(ncin):
                            nc.tensor.matmul(
                                ps[:],
                                w1T_sb[:, k, oc * P:(oc + 1) * P],
                                x_sb[:, k, f * FB:(f + 1) * FB],
                                start=(k == 0),
                                stop=(k == ncin - 1),
                            )
                        nc.vector.tensor_scalar_add(
                            wf_sb[:, oc, f * FB:(f + 1) * FB],
                            ps[:],
                            bias_sb[:, C_B1 + oc:C_B1 + oc + 1],
                        )
                for oc in range(nic):
                    for f in range(nfb):
                        ps = psMM.tile([P, FB], f32, tag="mm")
                        for k in range(ncin):
                            nc.tensor.matmul(
                                ps[:],
                                nodeT_sb[:, k, oc * P:(oc + 1) * P],
                                x_sb[:, k, f * FB:(f + 1) * FB],
                                start=(k == 0),
                                stop=(k == ncin - 1),
                            )
                        nc.vector.tensor_scalar_add(
                            ndT_sb[:, oc, f * FB:(f + 1) * FB],
                            ps[:],
                            bias_sb[:, C_NB + oc:C_NB + oc + 1],
                        )
                # nodes_sb[j, :] rows via PE transposes of ndT
                for j in range(nkn):
                    pst = psTR.tile([P, FB], f16, tag="trp")
                    for c in range(nic):
                        nc.tensor.transpose(
                            pst[:, c * P:(c + 1) * P],
                            ndT_sb[:, c, j * P:(j + 1) * P],
                            identh[:],
                        )
                    nc.vector.tensor_copy(nodes_sb[:, j, :], pst[:, 0:ic])

                # ---------------- Phase B: iw + b0 ----------------
                for pc in range(nkn):
                    for f in range(nfb):
                        ps = psMM.tile([P, FB], f32, tag="mm")
                        for k in range(nic):
                            nc.tensor.matmul(
                                ps[:],
                                wf_sb[:, k, pc * P:(pc + 1) * P],
                                wf_sb[:, k, f * FB:(f + 1) * FB],
                                start=(k == 0),
                                stop=(k == nic - 1),
                            )
                        b0t = evict.tile([P, FB], fp8, tag="b0t")
                        nc.vector.tensor_scalar(
                            b0t[:], ps[:], thr, None, op0=ALU.is_ge
                        )
                        nc.sync.dma_start(
                            b0q[pc * P:(pc + 1) * P, f * FB:(f + 1) * FB], b0t[:]
                        )
                        if pc * P < r:
                            iwt = evict.tile([P, FB], f32, tag="iwt")
                            nc.scalar.activation(iwt[:], ps[:], AF.Copy)
                            nc.sync.dma_start(
                                iwq[pc * P:(pc + 1) * P, f * FB:(f + 1) * FB],
                                iwt[:],
                            )

            # ------- Phase C: bh2 = b0 @ b0 (fp8 DoubleRow, exact) -------
            # + hop0 row-tiles, split-stage, one block of skew
            with (
                tc.tile_pool(name="pc_lhs", bufs=1) as pcl,
                tc.tile_pool(name="pc_rhs", bufs=2) as pcr,
                tc.tile_pool(name="evCp", bufs=2) as evict,
                tc.tile_pool(name="pe0", bufs=2) as pe0,
            ):
                RQH = nrt // 2
                pend = None
                for rqh in range(2):
                    lh = pcl.tile([P, nkn, RQH * P], fp8, tag="lhC")
                    nc.sync.dma_start(
                        lh[:],
                        b0q[:, rqh * RQH * P:(rqh + 1) * RQH * P].rearrange(
                            "(k p) q -> p k q", p=P
                        ),
                    )
                    for mc in range(nfb):
                        rt_ = pcr.tile([P, nkn, FB], fp8, tag="rhsC")
                        nc.sync.dma_start(
                            rt_[:],
                            b0q[:, mc * FB:(mc + 1) * FB].rearrange(
                                "(k p) q -> p k q", p=P
                            ),
                        )
                        for rq in range(RQH):
                            ps = psMM.tile([P, FB], f32, tag="mm")
                            for k in range(0, nkn, 2):
                                nc.tensor.matmul(
                                    ps[:],
                                    lh[:, k:k + 2, rq * P:(rq + 1) * P],
                                    rt_[:, k:k + 2, :],
                                    start=(k == 0),
                                    stop=(k == nkn - 2),
                                    perf_mode=DR,
                                )
                            rg = (rqh * RQH + rq) * P
                            ev32 = evict.tile([P, FB], f32, tag="evC32")
                            nc.vector.tensor_copy(ev32[:], ps[:])
                            nc.sync.dma_start(
                                bh2[rg:rg + P, mc * FB:(mc + 1) * FB], ev32[:]
                            )
                            ev = evict.tile([P, FB], f16, tag="evC")
                            nc.vector.tensor_copy(ev[:], ps[:])
                            pst = psTR.tile([P, FB], f16, tag="trp")
                            for q in range(FB // P):
                                nc.tensor.transpose(
                                    pst[:, q * P:(q + 1) * P],
                                    ev[:, q * P:(q + 1) * P],
                                    identh[:],
                                )
                            tT_ = evict.tile([P, FB // P, P], f16, tag="tTC")
                            for q in range(FB // P):
                                nc.vector.tensor_copy(
                                    tT_[:, q, :], pst[:, q * P:(q + 1) * P]
                                )
                            nc.sync.dma_start(
                                bh2T[
                                    mc * FB:(mc + 1) * FB, rg:rg + P
                                ].rearrange("(j p) q -> p j q", p=P),
                                tT_[:],
                            )
                        if pend is not None:
                            hop_stage2(pend)
                            pend = None
                        rt0 = rqh * nfb + mc
                        if rt0 < nrt:
                            pend = hop_stage1(pe0, 0, rt0)
                if pend is not None:
                    hop_stage2(pend)

            # ------- Phase D: bh3 = bh2 @ b0 (f16 x fp8, exact f32) -------
            # + hop1 (this rh) and hop2 (previous rh) row-tiles interleaved
            RH = min(512, r)
            nrh = r // RH
            nrq = RH // P
            with (
                tc.tile_pool(name="pd_lhs", bufs=1) as pdl,
                tc.tile_pool(name="pd_rhs", bufs=2) as pdr,
                tc.tile_pool(name="evDp", bufs=2) as evict,
                tc.tile_pool(name="peD", bufs=2) as peD,
            ):
                for rh in range(nrh):
                    lh = pdl.tile([P, nkn, RH], f16, tag="lhD")
                    nc.sync.dma_start(
                        lh[:],
                        bh2T[:, rh * RH:(rh + 1) * RH].rearrange(
                            "(k p) q -> p k q", p=P
                        ),
                    )
                    for mc in range(nfb):
                        rt_ = pdr.tile([P, nkn, FB], fp8, tag="rhsD")
                        nc.sync.dma_start(
                            rt_[:],
                            b0q[:, mc * FB:(mc + 1) * FB].rearrange(
                                "(k p) q -> p k q", p=P
                            ),
                        )
                        for rq in range(nrq):
                            ps = psMM.tile([P, FB], f32, tag="mm")
                            for k in range(nkn):
                                nc.tensor.matmul(
                                    ps[:],
                                    lh[:, k, rq * P:(rq + 1) * P],
                                    rt_[:, k, :],
                                    start=(k == 0),
                                    stop=(k == nkn - 1),
                                )
                            ev = evict.tile([P, FB], f32, tag="evD")
                            nc.vector.tensor_copy(ev[:], ps[:])
                            rg = rh * RH + rq * P
                            nc.sync.dma_start(
                                bh3[rg:rg + P, mc * FB:(mc + 1) * FB], ev[:]
                            )
                    iters = [(1, rh * nrq + j) for j in range(nrq)]
                    if rh > 0:
                        iters += [(2, (rh - 1) * nrq + j) for j in range(nrq)]
                    pend = None
                    for i, rt in iters:
                        st = hop_stage1(peD, i, rt)
                        if pend is not None:
                            hop_stage2(pend)
                        pend = st
                    if pend is not None:
                        hop_stage2(pend)
                # last rh block of hop2
                pend = None
                for j in range(nrq):
                    st = hop_stage1(peD, 2, (nrh - 1) * nrq + j)
                    if pend is not None:
                        hop_stage2(pend)
                    pend = st
                if pend is not None:
                    hop_stage2(pend)

            # ---------------- Phase F: fuse + residual ----------------
            FR = min(512, r)
            nrf = r // FR
            with (
                tc.tile_pool(name="pf", bufs=1) as pf,
                tc.tile_pool(name="evFp", bufs=2) as evict,
            ):
                xp_sb = pf.tile([P, nic, r], f16, tag="xp_sb")
                fuseT_sb = pf.tile([P, nhc, ic], f16, tag="fuseT_sb")
                nc.sync.dma_start(
                    fuseT_sb[:], fuseT[:, :].rearrange("(k p) o -> p k o", p=P)
                )
                resT_sb = pf.tile([P, 2 * nic, cout], f16, tag="resT_sb")
                nc.sync.dma_start(
                    resT_sb[:], resT[:, :].rearrange("(k p) o -> p k o", p=P)
                )
                xres_sb = pf.tile([P, nxc, r], f16, tag="xres_sb")
                nc.sync.dma_start(
                    xres_sb[:], xb[0:ic, 0:r].rearrange("(k p) q -> p k q", p=P)
                )
                for rf in range(nrf):
                    for o in range(nic):
                        ps = psMM.tile([P, FR], f32, tag="mm")
                        for k in range(nhc):
                            nc.tensor.matmul(
                                ps[:],
                                fuseT_sb[:, k, o * P:(o + 1) * P],
                                hops_sb[:, k, rf * FR:(rf + 1) * FR],
                                start=(k == 0),
                                stop=(k == nhc - 1),
                            )
                        nc.vector.tensor_scalar_add(
                            xp_sb[:, o, rf * FR:(rf + 1) * FR],
                            ps[:],
                            bias_sb[:, C_FUSE + o:C_FUSE + o + 1],
                        )

                for o in range(ncout):
                    for rf in range(nrf):
                        ps = psMM.tile([P, FR], f32, tag="mm")
                        for k in range(nxc):
                            nc.tensor.matmul(
                                ps[:],
                                resT_sb[:, k, o * P:(o + 1) * P],
                                xres_sb[:, k, rf * FR:(rf + 1) * FR],
                                start=(k == 0),
                                stop=False,
                            )
                        for k in range(nic):
                            nc.tensor.matmul(
                                ps[:],
                                resT_sb[:, nxc + k, o * P:(o + 1) * P],
                                xp_sb[:, k, rf * FR:(rf + 1) * FR],
                                start=False,
                                stop=(k == nic - 1),
                            )
                        ev = evict.tile([P, FR], f32, tag="evF")
                        nc.vector.tensor_scalar_add(
                            ev[:], ps[:], bias_sb[:, C_RES + o:C_RES + o + 1]
                        )
                        nc.sync.dma_start(
                            out[o * P:(o + 1) * P, rf * FR:(rf + 1) * FR], ev[:]
                        )

    nc.compile()
    return nc


def _host_prep(inputs, cin, ic, n, r, hop, eps):
    """Fold BN into weights; build per-core input maps."""

    def f32(a):
        return np.ascontiguousarray(np.asarray(a, dtype=np.float32))

    x = f32(inputs["x"])
    B = x.shape[0]
    xf = x.reshape(B, cin, n)

    s4 = float(ic) ** -0.25
    inv1 = 1.0 / np.sqrt(f32(inputs["bn1_v"]) + eps) * f32(inputs["bn1_g"])
    w1_eff = (inv1[:, None] * f32(inputs["w1_w"])) * s4
    b1_eff = (f32(inputs["w1_b"]) * inv1 + f32(inputs["bn1_b"])
              - f32(inputs["bn1_m"]) * inv1) * s4

    invf = 1.0 / np.sqrt(f32(inputs["bnf_v"]) + eps) * f32(inputs["bnf_g"])
    fuse_eff = invf[:, None] * f32(inputs["fuse_w"])
    fuse_b_eff = (f32(inputs["fuse_b"]) * invf + f32(inputs["bnf_b"])
                  - f32(inputs["bnf_m"]) * invf)

    invr = 1.0 / np.sqrt(f32(inputs["bnr_v"]) + eps) * f32(inputs["bnr_g"])
    res_eff = invr[:, None] * f32(inputs["res_w"])
    res_b_eff = (f32(inputs["res_b"]) * invr + f32(inputs["bnr_b"])
                 - f32(inputs["bnr_m"]) * invr)

    delta = float(np.asarray(inputs["delta"]).reshape(-1)[0])
    if delta <= 0.0:
        thr = -3.0e38
    elif delta >= 1.0:
        thr = 3.0e38
    else:
        thr = float(np.log(delta / (1.0 - delta)))

    nic = ic // P
    ncout = cin // P
    nbias = nic + nic + hop * nic + nic + ncout
    bias_pack = np.zeros((P, nbias), np.float32)
    col = 0
    for oc in range(nic):
        bias_pack[:, col] = b1_eff[oc * P:(oc + 1) * P]
        col += 1
    node_b = f32(inputs["node_b"])
    for oc in range(nic):
        bias_pack[:, col] = node_b[oc * P:(oc + 1) * P]
        col += 1
    hop_b = f32(inputs["hop_b"])
    for i in range(hop):
        for oc in range(nic):
            bias_pack[:, col] = hop_b[i, oc * P:(oc + 1) * P]
            col += 1
    for oc in range(nic):
        bias_pack[:, col] = fuse_b_eff[oc * P:(oc + 1) * P]
        col += 1
    for oc in range(ncout):
        bias_pack[:, col] = res_b_eff[oc * P:(oc + 1) * P]
        col += 1

    f16 = np.float16
    shared = {
        "w1T": np.ascontiguousarray(w1_eff.T.astype(f16)),
        "nodeT": np.ascontiguousarray(f32(inputs["node_w"]).T.astype(f16)),
        "hopT": np.ascontiguousarray(
            f32(inputs["hop_w"]).transpose(0, 2, 1).astype(f16)
        ),
        "fuseT": np.ascontiguousarray(fuse_eff.T.astype(f16)),
        "resT": np.ascontiguousarray(res_eff.T.astype(f16)),
        "biases": bias_pack,
    }

    n_cores = (B * n) // r
    halves = n // r
    in_maps = []
    for c in range(n_cores):
        b, h = c // halves, c % halves
        perm = (np.arange(n) + h * r) % n
        m = dict(shared)
        m["xb"] = np.ascontiguousarray(xf[b][:, perm].astype(f16))
        in_maps.append(m)
    return in_maps, thr


_BUILD_CACHE = {}


def kernel(**inputs):
    from concourse import bass_utils

    cin, ic, hop, eps = 512, 256, 3, 1e-5
    x = np.asarray(inputs["x"])
    B, _, H, W = x.shape
    n = H * W
    n_cores = 8
    r = (B * n) // n_cores
    halves = n // r

    in_maps, thr = _host_prep(inputs, cin, ic, n, r, hop, eps)

    key = (cin, ic, n, r, hop, thr)
    if key not in _BUILD_CACHE:
        _BUILD_CACHE[key] = _build(cin, ic, n, r, hop, thr)
    nc = _BUILD_CACHE[key]

    res = bass_utils.run_bass_kernel_spmd(nc, in_maps, core_ids=list(range(n_cores)))

    out = np.empty((B, cin, n), np.float32)
    for c in range(n_cores):
        b, h = c // halves, c % halves
        out[b][:, h * r:(h + 1) * r] = res.results[c]["out"]
    return out.reshape(B, cin, H, W).astype(x.dtype)


# revision 33
# speedup vs baseline: 1.1284x; 1.0778x over previous
"""Trainium2 Bass kernel for nn_HA_unit (gnn_message_passing).

Math (per batch b, N = H*W spatial positions):
  wfeat = BN1(w1 @ x)                       [IC, N]   (BN folded on host)
  iw    = wfeat^T wfeat * IC^-0.5           [N, N]    symmetric
  nodes = node_w @ x + node_b               [N, IC]
  b0    = (sigmoid(iw) >= delta)            [N, N]    binary, symmetric
  bh_k  = b0^k  (k = 1, 2, 3)               exact integer counts
  hop_k = hopw_k @ (softmax(bh_k o iw) @ nodes)^T + hopb_k
  xp    = BNf(fuse_w @ concat(hops))
  out   = BNr(res_w @ concat(x[:IC], xp))

Sharding: 8 cores = 4 batches x 2 halves of N. Core (b, h) receives x[b]
with spatial positions rolled by h*N/2 so that its rows are always 0..N/2-1
(identical SPMD program, data-only difference). Each core computes the full
symmetric b0 locally (no collectives); the b0^2 / b0^3 matmuls are sharded
by output rows.

Performance structure:
  - all dense matmuls on f16 operands (1 cyc/row); fp8 DoubleRow
    (0.5 cyc/row) for b0 @ b0.  b0 is exact in fp8, bh2 (< 2048) exact
    in f16, bh3 exact in f32 PSUM.
  - single 4-deep PSUM pool for every accumulation group keeps TensorE
    several groups ahead of the evictions (HAM stays un-throttled).
  - softmax hop row-tiles run split-stage (stage1 = DMA+logits+exp on
    DVE/POOL/ACT, stage2 = transposes + matmuls on PE) with a one-block
    skew, interleaved into the big matmul phases: hop0 inside C, hop1
    and hop2 inside D.  Only fuse+residual remain as a tail.
  - nodes / hops stay SBUF-resident; evictions ride DVE, not ACT.
"""

import sys

sys.path.insert(0, "/opt/trn_rl_repo")

import numpy as np

P = 128


def _build(cin, ic, n, r, hop, thr):
    from concourse import bass, tile, bacc
    import concourse.mybir as mybir
    from concourse.masks import make_identity

    f32 = mybir.dt.float32
    f16 = mybir.dt.float16
    fp8 = mybir.dt.float8e4
    AF = mybir.ActivationFunctionType
    ALU = mybir.AluOpType
    AX = mybir.AxisListType
    DR = mybir.MatmulPerfMode.DoubleRow

    ncin = cin // P          # K-chunks over input channels
    nic = ic // P            # chunks over inter channels
    nkn = n // P             # K-chunks over N
    nrt = r // P             # our row tiles
    FB = min(512, n)         # free-dim blocking
    nfb = n // FB
    hc = hop * ic
    nhc = hc // P
    cout = cin
    ncout = cout // P
    nxc = ic // P            # x residual slice chunks (x[:ic])

    # bias_pack columns: [b1(nic) | nodeb(nic) | hop(hop*nic) | fuse(nic) | res(ncout)]
    C_B1 = 0
    C_NB = C_B1 + nic
    C_HOP = C_NB + nic
    C_FUSE = C_HOP + hop * nic
    C_RES = C_FUSE + nic
    NBIAS = C_RES + ncout

    nc = bacc.Bacc("TRN2", target_bir_lowering=False, debug=False)

    xb = nc.dram_tensor("xb", [cin, n], f16, kind="ExternalInput")
    w1T = nc.dram_tensor("w1T", [cin, ic], f16, kind="ExternalInput")
    nodeT = nc.dram_tensor("nodeT", [cin, ic], f16, kind="ExternalInput")
    hopT = nc.dram_tensor("hopT", [hop, ic, ic], f16, kind="ExternalInput")
    fuseT = nc.dram_tensor("fuseT", [hc, ic], f16, kind="ExternalInput")
    resT = nc.dram_tensor("resT", [2 * ic, cout], f16, kind="ExternalInput")
    biases = nc.dram_tensor("biases", [P, NBIAS], f32, kind="ExternalInput")
    out = nc.dram_tensor("out", [cout, r], f32, kind="ExternalOutput")

    with tile.TileContext(nc) as tc:
        with (
            tc.tile_pool(name="dram", bufs=1, space="DRAM") as dpool,
            tc.tile_pool(name="consts", bufs=1) as consts,
            tc.tile_pool(name="res", bufs=1) as resid,
            tc.tile_pool(name="psMM", bufs=4, space="PSUM") as psMM,
            tc.tile_pool(name="psTR", bufs=3, space="PSUM") as psTR,
        ):
            b0q = dpool.tile([n, n], fp8, tag="b0q")
            iwq = dpool.tile([r, n], f32, tag="iwq")
            bh2 = dpool.tile([r, n], f32, tag="bh2")
            bh2T = dpool.tile([n, r], f16, tag="bh2T")
            bh3 = dpool.tile([r, n], f32, tag="bh3")

            identh = consts.tile([P, P], f16, tag="identh")
            make_identity(nc, identh[:])
            bias_sb = consts.tile([P, NBIAS], f32, tag="bias_sb")
            nc.sync.dma_start(bias_sb[:], biases[:])

            # persistent SBUF residents
            nodes_sb = resid.tile([P, nkn, ic], f16, tag="nodes_sb")
            hops_sb = resid.tile([P, hop * nic, r], f16, tag="hops_sb")
            hopT_sb = resid.tile([P, hop, nic, ic], f16, tag="hopT_sb")
            for i in range(hop):
                nc.sync.dma_start(
                    hopT_sb[:, i, :, :],
                    hopT[i, :, :].rearrange("(k p) o -> p k o", p=P),
                )

            # ---- softmax hop row-tile, split in two stages ----
            # stage1: load iw row + bh row, logits, rowmax, exp (+rowsum)
            # stage2: transpose s, t = s @ nodes (scaled 1/Z), hop conv
            def hop_stage1(pool, i, rt):
                iwt = pool.tile([P, n], f32, tag="E_iw")
                nc.sync.dma_start(iwt[:], iwq[rt * P:(rt + 1) * P, :])
                if i == 0:
                    bht = pool.tile([P, n], fp8, tag="E_bh0")
                    nc.sync.dma_start(bht[:], b0q[rt * P:(rt + 1) * P, :])
                    nc.vector.tensor_mul(iwt[:], iwt[:], bht[:])
                else:
                    src = bh2 if i == 1 else bh3
                    bht = pool.tile([P, n], f32, tag="E_bh", bufs=2)
                    nc.sync.dma_start(bht[:], src[rt * P:(rt + 1) * P, :])
                    nc.gpsimd.tensor_mul(iwt[:], iwt[:], bht[:])
                nmax = pool.tile([P, 1], f32, tag="E_nm")
                nc.vector.tensor_reduce(
                    nmax[:], iwt[:], axis=AX.X, op=ALU.max, negate=True
                )
                zt = pool.tile([P, 1], f32, tag="E_z")
                pt_ = pool.tile([P, n], f16, tag="E_p")
                nc.scalar.activation(
                    pt_[:], iwt[:], AF.Exp, bias=nmax[:], accum_out=zt[:]
                )
                rz = pool.tile([P, 1], f32, tag="E_rz")
                nc.vector.reciprocal(rz[:], zt[:])
                return (pool, i, rt, pt_, rz)

            def hop_stage2(state):
                pool, i, rt, pt_, rz = state
                sT = pool.tile([P, nfb, FB], f16, tag="E_sT", bufs=1)
                for jb in range(nfb):
                    pst = psTR.tile([P, FB], f16, tag="trp")
                    for q in range(FB // P):
                        nc.tensor.transpose(
                            pst[:, q * P:(q + 1) * P],
                            pt_[:, jb * FB + q * P:jb * FB + (q + 1) * P],
                            identh[:],
                        )
                    nc.vector.tensor_copy(sT[:, jb, :], pst[:])
                tps = psMM.tile([P, FB], f32, tag="mm")
                for j in range(nkn):
                    nc.tensor.matmul(
                        tps[:, 0:ic],
                        sT[:, j // (FB // P), (j % (FB // P)) * P:
                           (j % (FB // P)) * P + P],
                        nodes_sb[:, j, :],
                        start=(j == 0),
                        stop=(j == nkn - 1),
                    )
                t_sb = pool.tile([P, ic], f16, tag="E_t")
                nc.vector.tensor_scalar_mul(t_sb[:], tps[:, 0:ic], rz[:])
                tT = pool.tile([P, nic, P], f16, tag="E_tT")
                for q in range(nic):
                    ptq = psTR.tile([P, P], f16, tag="trp")
                    nc.tensor.transpose(
                        ptq[:], t_sb[:, q * P:(q + 1) * P], identh[:]
                    )
                    nc.vector.tensor_copy(tT[:, q, :], ptq[:])
                for o in range(nic):
                    ph = psTR.tile([P, P], f32, tag="trp")
                    for c in range(nic):
                        nc.tensor.matmul(
                            ph[:],
                            hopT_sb[:, i, c, o * P:(o + 1) * P],
                            tT[:, c, :],
                            start=(c == 0),
                            stop=(c == nic - 1),
                        )
                    nc.vector.tensor_scalar_add(
                        hops_sb[:, i * nic + o, rt * P:(rt + 1) * P],
                        ph[:],
                        bias_sb[
                            :, C_HOP + i * nic + o:C_HOP + i * nic + o + 1
                        ],
                    )

            # ---------------- Phase A: wfeat + nodes ----------------
            with (
                tc.tile_pool(name="pa", bufs=1) as pa,
                tc.tile_pool(name="evA", bufs=2) as evict,
            ):
                x_sb = pa.tile([P, ncin, n], f16, tag="x_sb")
                nc.sync.dma_start(
                    x_sb[:], xb[:, :].rearrange("(k p) n -> p k n", p=P)
                )
                w1T_sb = pa.tile([P, ncin, ic], f16, tag="w1T_sb")
                nc.sync.dma_start(
                    w1T_sb[:], w1T[:, :].rearrange("(k p) o -> p k o", p=P)
                )
                nodeT_sb = pa.tile([P, ncin, ic], f16, tag="nodeT_sb")
                nc.sync.dma_start(
                    nodeT_sb[:], nodeT[:, :].rearrange("(k p) o -> p k o", p=P)
                )
                wf_sb = pa.tile([P, nic, n], f16, tag="wf_sb")
                ndT_sb = pa.tile([P, nic, n], f16, tag="ndT_sb")

                for oc in range(nic):
                    for f in range(nfb):
                        ps = psMM.tile([P, FB], f32, tag="mm")
                        for k in range(ncin):
                            nc.tensor.matmul(
                                ps[:],
                                w1T_sb[:, k, oc * P:(oc + 1) * P],
                                x_sb[:, k, f * FB:(f + 1) * FB],
                                start=(k == 0),
                                stop=(k == ncin - 1),
                            )
                        nc.vector.tensor_scalar_add(
                            wf_sb[:, oc, f * FB:(f + 1) * FB],
                            ps[:],
                            bias_sb[:, C_B1 + oc:C_B1 + oc + 1],
                        )
                for oc in range(nic):
                    for f in range(nfb):
                        ps = psMM.tile([P, FB], f32, tag="mm")
                        for k in range(ncin):
                            nc.tensor.matmul(
                                ps[:],
                                nodeT_sb[:, k, oc * P:(oc + 1) * P],
                                x_sb[:, k, f * FB:(f + 1) * FB],
                                start=(k == 0),
                                stop=(k == ncin - 1),
                            )
                        nc.vector.tensor_scalar_add(
                            ndT_sb[:, oc, f * FB:(f + 1) * FB],
                            ps[:],
                            bias_sb[:, C_NB + oc:C_NB + oc + 1],
                        )
                # nodes_sb[j, :] rows via PE transposes of ndT
                for j in range(nkn):
                    pst = psTR.tile([P, FB], f16, tag="trp")
                    for c in range(nic):
                        nc.tensor.transpose(
                            pst[:, c * P:(c + 1) * P],
                            ndT_sb[:, c, j * P:(j + 1) * P],
                            identh[:],
                        )
                    nc.vector.tensor_copy(nodes_sb[:, j, :], pst[:, 0:ic])

                # ---------------- Phase B: iw + b0 ----------------
                # f-outer so the left half of b0q completes early and Phase
                # C's stationary-operand load can prefetch during B's tail.
                for f in range(nfb):
                    for pc in range(nkn):
                        ps = psMM.tile([P, FB], f32, tag="mm")
                        for k in range(nic):
                            nc.tensor.matmul(
                                ps[:],
                                wf_sb[:, k, pc * P:(pc + 1) * P],
                                wf_sb[:, k, f * FB:(f + 1) * FB],
                                start=(k == 0),
                                stop=(k == nic - 1),
                            )
                        b0t = evict.tile([P, FB], fp8, tag="b0t", bufs=6)
                        nc.vector.tensor_scalar(
                            b0t[:], ps[:], thr, None, op0=ALU.is_ge
                        )
                        nc.sync.dma_start(
                            b0q[pc * P:(pc + 1) * P, f * FB:(f + 1) * FB], b0t[:]
                        )
                        if pc * P < r:
                            iwt = evict.tile([P, FB], f32, tag="iwt", bufs=4)
                            nc.scalar.activation(iwt[:], ps[:], AF.Copy)
                            nc.sync.dma_start(
                                iwq[pc * P:(pc + 1) * P, f * FB:(f + 1) * FB],
                                iwt[:],
                            )

            # ------- Phase C: bh2 = b0 @ b0 (fp8 DoubleRow, exact) -------
            # + hop0 row-tiles, split-stage, one block of skew
            with (
                tc.tile_pool(name="pc_lhs", bufs=1) as pcl,
                tc.tile_pool(name="pc_rhs", bufs=2) as pcr,
                tc.tile_pool(name="evCp", bufs=2) as evict,
                tc.tile_pool(name="pe0", bufs=2) as pe0,
            ):
                RQH = nrt // 2
                pend = None
                for rqh in range(2):
                    lh = pcl.tile([P, nkn, RQH * P], fp8, tag="lhC")
                    nc.sync.dma_start(
                        lh[:],
                        b0q[:, rqh * RQH * P:(rqh + 1) * RQH * P].rearrange(
                            "(k p) q -> p k q", p=P
                        ),
                    )
                    for mc in range(nfb):
                        rt_ = pcr.tile([P, nkn, FB], fp8, tag="rhsC")
                        nc.sync.dma_start(
                            rt_[:],
                            b0q[:, mc * FB:(mc + 1) * FB].rearrange(
                                "(k p) q -> p k q", p=P
                            ),
                        )
                        for rq in range(RQH):
                            ps = psMM.tile([P, FB], f32, tag="mm")
                            for k in range(0, nkn, 2):
                                nc.tensor.matmul(
                                    ps[:],
                                    lh[:, k:k + 2, rq * P:(rq + 1) * P],
                                    rt_[:, k:k + 2, :],
                                    start=(k == 0),
                                    stop=(k == nkn - 2),
                                    perf_mode=DR,
                                )
                            rg = (rqh * RQH + rq) * P
                            ev32 = evict.tile([P, FB], f32, tag="evC32", bufs=3)
                            nc.vector.tensor_copy(ev32[:], ps[:])
                            nc.sync.dma_start(
                                bh2[rg:rg + P, mc * FB:(mc + 1) * FB], ev32[:]
                            )
                            ev = evict.tile([P, FB], f16, tag="evC", bufs=3)
                            nc.vector.tensor_copy(ev[:], ps[:])
                            pst = psTR.tile([P, FB], f16, tag="trp")
                            for q in range(FB // P):
                                nc.tensor.transpose(
                                    pst[:, q * P:(q + 1) * P],
                                    ev[:, q * P:(q + 1) * P],
                                    identh[:],
                                )
                            tT_ = evict.tile([P, FB // P, P], f16, tag="tTC",
                                             bufs=3)
                            for q in range(FB // P):
                                nc.vector.tensor_copy(
                                    tT_[:, q, :], pst[:, q * P:(q + 1) * P]
                                )
                            nc.sync.dma_start(
                                bh2T[
                                    mc * FB:(mc + 1) * FB, rg:rg + P
                                ].rearrange("(j p) q -> p j q", p=P),
                                tT_[:],
                            )
                        if pend is not None:
                            hop_stage2(pend)
                            pend = None
                        rt0 = rqh * nfb + mc
                        if rt0 < nrt:
                            pend = hop_stage1(pe0, 0, rt0)
                if pend is not None:
                    hop_stage2(pend)

            # ------- Phase D: bh3 = bh2 @ b0 (f16 x fp8, exact f32) -------
            # + hop1 (this rh) and hop2 (previous rh) row-tiles interleaved
            RH = min(512, r)
            nrh = r // RH
            nrq = RH // P
            with (
                tc.tile_pool(name="pd_lhs", bufs=1) as pdl,
                tc.tile_pool(name="pd_rhs", bufs=2) as pdr,
                tc.tile_pool(name="evDp", bufs=2) as evict,
                tc.tile_pool(name="peD", bufs=2) as peD,
            ):
                for rh in range(nrh):
                    lh = pdl.tile([P, nkn, RH], f16, tag="lhD")
                    nc.sync.dma_start(
                        lh[:],
                        bh2T[:, rh * RH:(rh + 1) * RH].rearrange(
                            "(k p) q -> p k q", p=P
                        ),
                    )
                    for mc in range(nfb):
                        rt_ = pdr.tile([P, nkn, FB], fp8, tag="rhsD")
                        nc.sync.dma_start(
                            rt_[:],
                            b0q[:, mc * FB:(mc + 1) * FB].rearrange(
                                "(k p) q -> p k q", p=P
                            ),
                        )
                        for rq in range(nrq):
                            ps = psMM.tile([P, FB], f32, tag="mm")
                            for k in range(nkn):
                                nc.tensor.matmul(
                                    ps[:],
                                    lh[:, k, rq * P:(rq + 1) * P],
                                    rt_[:, k, :],
                                    start=(k == 0),
                                    stop=(k == nkn - 1),
                                )
                            ev = evict.tile([P, FB], f32, tag="evD", bufs=4)
                            nc.vector.tensor_copy(ev[:], ps[:])
                            rg = rh * RH + rq * P
                            nc.sync.dma_start(
                                bh3[rg:rg + P, mc * FB:(mc + 1) * FB], ev[:]
                            )
                    iters = [(1, rh * nrq + j) for j in range(nrq)]
                    if rh > 0:
                        iters += [(2, (rh - 1) * nrq + j) for j in range(nrq)]
                    pend = None
                    for i, rt in iters:
                        st = hop_stage1(peD, i, rt)
                        if pend is not None:
                            hop_stage2(pend)
                        pend = st
                    if pend is not None:
                        hop_stage2(pend)
                # last rh block of hop2
                pend = None
                for j in range(nrq):
                    st = hop_stage1(peD, 2, (nrh - 1) * nrq + j)
                    if pend is not None:
                        hop_stage2(pend)
                    pend = st
                if pend is not None:
                    hop_stage2(pend)

            # ---------------- Phase F: fuse + residual ----------------
            FR = min(512, r)
            nrf = r // FR
            with (
                tc.tile_pool(name="pf", bufs=1) as pf,
                tc.tile_pool(name="evFp", bufs=2) as evict,
            ):
                xp_sb = pf.tile([P, nic, r], f16, tag="xp_sb")
                fuseT_sb = pf.tile([P, nhc, ic], f16, tag="fuseT_sb")
                nc.sync.dma_start(
                    fuseT_sb[:], fuseT[:, :].rearrange("(k p) o -> p k o", p=P)
                )
                resT_sb = pf.tile([P, 2 * nic, cout], f16, tag="resT_sb")
                nc.sync.dma_start(
                    resT_sb[:], resT[:, :].rearrange("(k p) o -> p k o", p=P)
                )
                xres_sb = pf.tile([P, nxc, r], f16, tag="xres_sb")
                nc.sync.dma_start(
                    xres_sb[:], xb[0:ic, 0:r].rearrange("(k p) q -> p k q", p=P)
                )
                for rf in range(nrf):
                    for o in range(nic):
                        ps = psMM.tile([P, FR], f32, tag="mm")
                        for k in range(nhc):
                            nc.tensor.matmul(
                                ps[:],
                                fuseT_sb[:, k, o * P:(o + 1) * P],
                                hops_sb[:, k, rf * FR:(rf + 1) * FR],
                                start=(k == 0),
                                stop=(k == nhc - 1),
                            )
                        nc.vector.tensor_scalar_add(
                            xp_sb[:, o, rf * FR:(rf + 1) * FR],
                            ps[:],
                            bias_sb[:, C_FUSE + o:C_FUSE + o + 1],
                        )

                for o in range(ncout):
                    for rf in range(nrf):
                        ps = psMM.tile([P, FR], f32, tag="mm")
                        for k in range(nxc):
                            nc.tensor.matmul(
                                ps[:],
                                resT_sb[:, k, o * P:(o + 1) * P],
                                xres_sb[:, k, rf * FR:(rf + 1) * FR],
                                start=(k == 0),
                                stop=False,
                            )
                        for k in range(nic):
                            nc.tensor.matmul(
                                ps[:],
                                resT_sb[:, nxc + k, o * P:(o + 1) * P],
                                xp_sb[:, k, rf * FR:(rf + 1) * FR],
                                start=False,
                                stop=(k == nic - 1),
                            )
                        ev = evict.tile([P, FR], f32, tag="evF")
                        nc.vector.tensor_scalar_add(
                            ev[:], ps[:], bias_sb[:, C_RES + o:C_RES + o + 1]
                        )
                        nc.sync.dma_start(
                            out[o * P:(o + 1) * P, rf * FR:(rf + 1) * FR], ev[:]
                        )

    nc.compile()
    return nc


def _host_prep(inputs, cin, ic, n, r, hop, eps):
    """Fold BN into weights; build per-core input maps."""

    def f32(a):
        return np.ascontiguousarray(np.asarray(a, dtype=np.float32))

    x = f32(inputs["x"])
    B = x.shape[0]
    xf = x.reshape(B, cin, n)

    s4 = float(ic) ** -0.25
    inv1 = 1.0 / np.sqrt(f32(inputs["bn1_v"]) + eps) * f32(inputs["bn1_g"])
    w1_eff = (inv1[:, None] * f32(inputs["w1_w"])) * s4
    b1_eff = (f32(inputs["w1_b"]) * inv1 + f32(inputs["bn1_b"])
              - f32(inputs["bn1_m"]) * inv1) * s4

    invf = 1.0 / np.sqrt(f32(inputs["bnf_v"]) + eps) * f32(inputs["bnf_g"])
    fuse_eff = invf[:, None] * f32(inputs["fuse_w"])
    fuse_b_eff = (f32(inputs["fuse_b"]) * invf + f32(inputs["bnf_b"])
                  - f32(inputs["bnf_m"]) * invf)

    invr = 1.0 / np.sqrt(f32(inputs["bnr_v"]) + eps) * f32(inputs["bnr_g"])
    res_eff = invr[:, None] * f32(inputs["res_w"])
    res_b_eff = (f32(inputs["res_b"]) * invr + f32(inputs["bnr_b"])
                 - f32(inputs["bnr_m"]) * invr)

    delta = float(np.asarray(inputs["delta"]).reshape(-1)[0])
    if delta <= 0.0:
        thr = -3.0e38
    elif delta >= 1.0:
        thr = 3.0e38
    else:
        thr = float(np.log(delta / (1.0 - delta)))

    nic = ic // P
    ncout = cin // P
    nbias = nic + nic + hop * nic + nic + ncout
    bias_pack = np.zeros((P, nbias), np.float32)
    col = 0
    for oc in range(nic):
        bias_pack[:, col] = b1_eff[oc * P:(oc + 1) * P]
        col += 1
    node_b = f32(inputs["node_b"])
    for oc in range(nic):
        bias_pack[:, col] = node_b[oc * P:(oc + 1) * P]
        col += 1
    hop_b = f32(inputs["hop_b"])
    for i in range(hop):
        for oc in range(nic):
            bias_pack[:, col] = hop_b[i, oc * P:(oc + 1) * P]
            col += 1
    for oc in range(nic):
        bias_pack[:, col] = fuse_b_eff[oc * P:(oc + 1) * P]
        col += 1
    for oc in range(ncout):
        bias_pack[:, col] = res_b_eff[oc * P:(oc + 1) * P]
        col += 1

    f16 = np.float16
    shared = {
        "w1T": np.ascontiguousarray(w1_eff.T.astype(f16)),
        "nodeT": np.ascontiguousarray(f32(inputs["node_w"]).T.astype(f16)),
        "hopT": np.ascontiguousarray(
            f32(inputs["hop_w"]).transpose(0, 2, 1).astype(f16)
        ),
        "fuseT": np.ascontiguousarray(fuse_eff.T.astype(f16)),
        "resT": np.ascontiguousarray(res_eff.T.astype(f16)),
        "biases": bias_pack,
    }

    n_cores = (B * n) // r
    halves = n // r
    in_maps = []
    for c in range(n_cores):
        b, h = c // halves, c % halves
        perm = (np.arange(n) + h * r) % n
        m = dict(shared)
        m["xb"] = np.ascontiguousarray(xf[b][:, perm].astype(f16))
        in_maps.append(m)
    return in_maps, thr


_BUILD_CACHE = {}


def kernel(**inputs):
    from concourse import bass_utils

    cin, ic, hop, eps = 512, 256, 3, 1e-5
    x = np.asarray(inputs["x"])
    B, _, H, W = x.shape
    n = H * W
    n_cores = 8
    r = (B * n) // n_cores
    halves = n // r

    in_maps, thr = _host_prep(inputs, cin, ic, n, r, hop, eps)

    key = (cin, ic, n, r, hop, thr)
    if key not in _BUILD_CACHE:
        _BUILD_CACHE[key] = _build(cin, ic, n, r, hop, thr)
    nc = _BUILD_CACHE[key]

    res = bass_utils.run_bass_kernel_spmd(nc, in_maps, core_ids=list(range(n_cores)))

    out = np.empty((B, cin, n), np.float32)
    for c in range(n_cores):
        b, h = c // halves, c % halves
        out[b][:, h * r:(h + 1) * r] = res.results[c]["out"]
    return out.reshape(B, cin, H, W).astype(x.dtype)


# revision 41
# speedup vs baseline: 1.1891x; 1.0538x over previous
"""Trainium2 Bass kernel for nn_HA_unit (gnn_message_passing).

Math (per batch b, N = H*W spatial positions):
  wfeat = BN1(w1 @ x)                       [IC, N]   (BN folded on host)
  iw    = wfeat^T wfeat * IC^-0.5           [N, N]    symmetric
  nodes = node_w @ x + node_b               [N, IC]
  b0    = (sigmoid(iw) >= delta)            [N, N]    binary, symmetric
  bh_k  = b0^k  (k = 1, 2, 3)               exact integer counts
  hop_k = hopw_k @ (softmax(bh_k o iw) @ nodes)^T + hopb_k
  xp    = BNf(fuse_w @ concat(hops))
  out   = BNr(res_w @ concat(x[:IC], xp))

Sharding: 8 cores = 4 batches x 2 halves of N. Core (b, h) receives x[b]
with spatial positions rolled by h*N/2 so that its rows are always 0..N/2-1
(identical SPMD program, data-only difference). Each core computes the full
symmetric b0 locally (no collectives); the b0^2 / b0^3 matmuls are sharded
by output rows.

Performance structure:
  - all dense matmuls on f16 operands (1 cyc/row); fp8 DoubleRow
    (0.5 cyc/row) for b0 @ b0.  b0 is exact in fp8, bh2 (< 2048) exact
    in f16, bh3 exact in f32 PSUM.
  - single 4-deep PSUM pool for every accumulation group keeps TensorE
    several groups ahead of the evictions (HAM stays un-throttled).
  - softmax hop row-tiles run split-stage (stage1 = DMA+logits+exp on
    DVE/POOL/ACT, stage2 = transposes + matmuls on PE) with a one-block
    skew, interleaved into the big matmul phases: hop0 inside C, hop1
    and hop2 inside D.  Only fuse+residual remain as a tail.
  - nodes / hops stay SBUF-resident; evictions ride DVE, not ACT.
"""

import sys

sys.path.insert(0, "/opt/trn_rl_repo")

import numpy as np

P = 128


def _build(cin, ic, n, r, hop, thr):
    from concourse import bass, tile, bacc
    import concourse.mybir as mybir
    from concourse.masks import make_identity

    f32 = mybir.dt.float32
    f16 = mybir.dt.float16
    fp8 = mybir.dt.float8e4
    AF = mybir.ActivationFunctionType
    ALU = mybir.AluOpType
    AX = mybir.AxisListType
    DR = mybir.MatmulPerfMode.DoubleRow

    ncin = cin // P          # K-chunks over input channels
    nic = ic // P            # chunks over inter channels
    nkn = n // P             # K-chunks over N
    nrt = r // P             # our row tiles
    FB = min(512, n)         # free-dim blocking
    nfb = n // FB
    hc = hop * ic
    nhc = hc // P
    cout = cin
    ncout = cout // P
    nxc = ic // P            # x residual slice chunks (x[:ic])

    # bias_pack columns: [b1(nic) | nodeb(nic) | hop(hop*nic) | fuse(nic) | res(ncout)]
    C_B1 = 0
    C_NB = C_B1 + nic
    C_HOP = C_NB + nic
    C_FUSE = C_HOP + hop * nic
    C_RES = C_FUSE + nic
    NBIAS = C_RES + ncout

    nc = bacc.Bacc("TRN2", target_bir_lowering=False, debug=False)

    xb = nc.dram_tensor("xb", [cin, n], f16, kind="ExternalInput")
    w1T = nc.dram_tensor("w1T", [cin, ic], f16, kind="ExternalInput")
    nodeT = nc.dram_tensor("nodeT", [cin, ic], f16, kind="ExternalInput")
    hopT = nc.dram_tensor("hopT", [hop, ic, ic], f16, kind="ExternalInput")
    fuseT = nc.dram_tensor("fuseT", [hc, ic], f16, kind="ExternalInput")
    resT = nc.dram_tensor("resT", [2 * ic, cout], f16, kind="ExternalInput")
    biases = nc.dram_tensor("biases", [P, NBIAS], f32, kind="ExternalInput")
    out = nc.dram_tensor("out", [cout, r], f32, kind="ExternalOutput")

    with tile.TileContext(nc) as tc:
        with (
            tc.tile_pool(name="dram", bufs=1, space="DRAM") as dpool,
            tc.tile_pool(name="consts", bufs=1) as consts,
            tc.tile_pool(name="res", bufs=1) as resid,
            tc.tile_pool(name="psMM", bufs=4, space="PSUM") as psMM,
            tc.tile_pool(name="psTR", bufs=3, space="PSUM") as psTR,
        ):
            b0q = dpool.tile([n, n], fp8, tag="b0q")
            iwq = dpool.tile([r, n], f32, tag="iwq")
            bh2 = dpool.tile([r, n], f32, tag="bh2")
            bh2T = dpool.tile([n, r], f16, tag="bh2T")
            bh3 = dpool.tile([r, n], f32, tag="bh3")

            identh = consts.tile([P, P], f16, tag="identh")
            make_identity(nc, identh[:])
            ident32 = consts.tile([P, P], f32, tag="ident32")
            make_identity(nc, ident32[:])
            bias_sb = consts.tile([P, NBIAS], f32, tag="bias_sb")
            nc.sync.dma_start(bias_sb[:], biases[:])

            # persistent SBUF residents
            nodes_sb = resid.tile([P, nkn, ic], f16, tag="nodes_sb")
            hops_sb = resid.tile([P, hop * nic, r], f16, tag="hops_sb")
            hopT_sb = resid.tile([P, hop, nic, ic], f16, tag="hopT_sb")
            for i in range(hop):
                nc.sync.dma_start(
                    hopT_sb[:, i, :, :],
                    hopT[i, :, :].rearrange("(k p) o -> p k o", p=P),
                )

            # ---- softmax hop row-tile, split in two stages ----
            # stage1: load iw row + bh row, logits, rowmax, exp (+rowsum)
            # stage2: transpose s, t = s @ nodes (scaled 1/Z), hop conv
            def hop_stage1(pool, i, rt):
                iwt = pool.tile([P, n], f32, tag="E_iw")
                nc.sync.dma_start(iwt[:], iwq[rt * P:(rt + 1) * P, :])
                if i == 0:
                    bht = pool.tile([P, n], fp8, tag="E_bh0")
                    nc.sync.dma_start(bht[:], b0q[rt * P:(rt + 1) * P, :])
                    nc.vector.tensor_mul(iwt[:], iwt[:], bht[:])
                else:
                    src = bh2 if i == 1 else bh3
                    bht = pool.tile([P, n], f32, tag="E_bh", bufs=2)
                    nc.sync.dma_start(bht[:], src[rt * P:(rt + 1) * P, :])
                    nc.gpsimd.tensor_mul(iwt[:], iwt[:], bht[:])
                nmax = pool.tile([P, 1], f32, tag="E_nm")
                nc.vector.tensor_reduce(
                    nmax[:], iwt[:], axis=AX.X, op=ALU.max, negate=True
                )
                zt = pool.tile([P, 1], f32, tag="E_z")
                pt_ = pool.tile([P, n], f16, tag="E_p")
                nc.scalar.activation(
                    pt_[:], iwt[:], AF.Exp, bias=nmax[:], accum_out=zt[:]
                )
                rz = pool.tile([P, 1], f32, tag="E_rz")
                nc.vector.reciprocal(rz[:], zt[:])
                return (pool, i, rt, pt_, rz)

            def hop_stage2(state):
                pool, i, rt, pt_, rz = state
                sT = pool.tile([P, nfb, FB], f16, tag="E_sT", bufs=1)
                for jb in range(nfb):
                    pst = psTR.tile([P, FB], f16, tag="trp")
                    for q in range(FB // P):
                        nc.tensor.transpose(
                            pst[:, q * P:(q + 1) * P],
                            pt_[:, jb * FB + q * P:jb * FB + (q + 1) * P],
                            identh[:],
                        )
                    nc.vector.tensor_copy(sT[:, jb, :], pst[:])
                tps = psMM.tile([P, FB], f32, tag="mm")
                for j in range(nkn):
                    nc.tensor.matmul(
                        tps[:, 0:ic],
                        sT[:, j // (FB // P), (j % (FB // P)) * P:
                           (j % (FB // P)) * P + P],
                        nodes_sb[:, j, :],
                        start=(j == 0),
                        stop=(j == nkn - 1),
                    )
                t_sb = pool.tile([P, ic], f16, tag="E_t")
                nc.vector.tensor_scalar_mul(t_sb[:], tps[:, 0:ic], rz[:])
                tT = pool.tile([P, nic, P], f16, tag="E_tT")
                for q in range(nic):
                    ptq = psTR.tile([P, P], f16, tag="trp")
                    nc.tensor.transpose(
                        ptq[:], t_sb[:, q * P:(q + 1) * P], identh[:]
                    )
                    nc.vector.tensor_copy(tT[:, q, :], ptq[:])
                for o in range(nic):
                    ph = psTR.tile([P, P], f32, tag="trp")
                    for c in range(nic):
                        nc.tensor.matmul(
                            ph[:],
                            hopT_sb[:, i, c, o * P:(o + 1) * P],
                            tT[:, c, :],
                            start=(c == 0),
                            stop=(c == nic - 1),
                        )
                    nc.vector.tensor_scalar_add(
                        hops_sb[:, i * nic + o, rt * P:(rt + 1) * P],
                        ph[:],
                        bias_sb[
                            :, C_HOP + i * nic + o:C_HOP + i * nic + o + 1
                        ],
                    )

            # ---------------- Phase A: wfeat + nodes ----------------
            with (
                tc.tile_pool(name="pa", bufs=1) as pa,
                tc.tile_pool(name="evA", bufs=2) as evict,
            ):
                x_sb = pa.tile([P, ncin, n], f16, tag="x_sb")
                nc.sync.dma_start(
                    x_sb[:], xb[:, :].rearrange("(k p) n -> p k n", p=P)
                )
                w1T_sb = pa.tile([P, ncin, ic], f16, tag="w1T_sb")
                nc.sync.dma_start(
                    w1T_sb[:], w1T[:, :].rearrange("(k p) o -> p k o", p=P)
                )
                nodeT_sb = pa.tile([P, ncin, ic], f16, tag="nodeT_sb")
                nc.sync.dma_start(
                    nodeT_sb[:], nodeT[:, :].rearrange("(k p) o -> p k o", p=P)
                )
                wf_sb = pa.tile([P, nic, n], f16, tag="wf_sb")
                ndT_sb = pa.tile([P, nic, n], f16, tag="ndT_sb")

                for oc in range(nic):
                    for f in range(nfb):
                        ps = psMM.tile([P, FB], f32, tag="mm")
                        for k in range(ncin):
                            nc.tensor.matmul(
                                ps[:],
                                w1T_sb[:, k, oc * P:(oc + 1) * P],
                                x_sb[:, k, f * FB:(f + 1) * FB],
                                start=(k == 0),
                                stop=(k == ncin - 1),
                            )
                        nc.vector.tensor_scalar_add(
                            wf_sb[:, oc, f * FB:(f + 1) * FB],
                            ps[:],
                            bias_sb[:, C_B1 + oc:C_B1 + oc + 1],
                        )
                for oc in range(nic):
                    for f in range(nfb):
                        ps = psMM.tile([P, FB], f32, tag="mm")
                        for k in range(ncin):
                            nc.tensor.matmul(
                                ps[:],
                                nodeT_sb[:, k, oc * P:(oc + 1) * P],
                                x_sb[:, k, f * FB:(f + 1) * FB],
                                start=(k == 0),
                                stop=(k == ncin - 1),
                            )
                        nc.vector.tensor_scalar_add(
                            ndT_sb[:, oc, f * FB:(f + 1) * FB],
                            ps[:],
                            bias_sb[:, C_NB + oc:C_NB + oc + 1],
                        )
                # nodes_sb[j, :] rows via PE transposes of ndT
                for j in range(nkn):
                    pst = psTR.tile([P, FB], f16, tag="trp")
                    for c in range(nic):
                        nc.tensor.transpose(
                            pst[:, c * P:(c + 1) * P],
                            ndT_sb[:, c, j * P:(j + 1) * P],
                            identh[:],
                        )
                    nc.vector.tensor_copy(nodes_sb[:, j, :], pst[:, 0:ic])

                # ---------------- Phase B: iw + b0 ----------------
                # f-outer so the left half of b0q completes early and Phase
                # C's stationary-operand load can prefetch during B's tail.
                for f in range(nfb):
                    for pc in range(nkn):
                        ps = psMM.tile([P, FB], f32, tag="mm")
                        for k in range(nic):
                            nc.tensor.matmul(
                                ps[:],
                                wf_sb[:, k, pc * P:(pc + 1) * P],
                                wf_sb[:, k, f * FB:(f + 1) * FB],
                                start=(k == 0),
                                stop=(k == nic - 1),
                            )
                        b0t = evict.tile([P, FB], fp8, tag="b0t", bufs=6)
                        nc.vector.tensor_scalar(
                            b0t[:], ps[:], thr, None, op0=ALU.is_ge
                        )
                        nc.sync.dma_start(
                            b0q[pc * P:(pc + 1) * P, f * FB:(f + 1) * FB], b0t[:]
                        )
                        if pc * P < r:
                            iwt = evict.tile([P, FB], f32, tag="iwt", bufs=4)
                            nc.scalar.activation(iwt[:], ps[:], AF.Copy)
                            nc.sync.dma_start(
                                iwq[pc * P:(pc + 1) * P, f * FB:(f + 1) * FB],
                                iwt[:],
                            )

            # ------- Phase C: bh2 = b0 @ b0 (fp8 DoubleRow, exact) -------
            # + hop0 row-tiles, split-stage, one block of skew
            with (
                tc.tile_pool(name="pc_lhs", bufs=1) as pcl,
                tc.tile_pool(name="pc_rhs", bufs=2) as pcr,
                tc.tile_pool(name="evCp", bufs=2) as evict,
                tc.tile_pool(name="pe0", bufs=2) as pe0,
            ):
                RQH = nrt // 2
                TPB = FB // P
                pend = None
                n_hop_emitted = 0

                def c_survivors(rqh, mc):
                    return [
                        rqh * RQH + rq
                        for rq in range(RQH)
                        if not (mc < nfb // 2
                                and TPB * mc + TPB - 1 < rqh * RQH + rq)
                    ]

                blocks = [(rqh, mc) for rqh in range(2) for mc in range(nfb)
                          if c_survivors(rqh, mc)]
                last_rqh = None
                for rqh, mc in blocks:
                    if rqh != last_rqh:
                        lh = pcl.tile([P, nkn, RQH * P], fp8, tag="lhC")
                        nc.sync.dma_start(
                            lh[:],
                            b0q[:, rqh * RQH * P:(rqh + 1) * RQH * P].rearrange(
                                "(k p) q -> p k q", p=P
                            ),
                        )
                        last_rqh = rqh
                    if True:
                        rt_ = pcr.tile([P, nkn, FB], fp8, tag="rhsC")
                        nc.sync.dma_start(
                            rt_[:],
                            b0q[:, mc * FB:(mc + 1) * FB].rearrange(
                                "(k p) q -> p k q", p=P
                            ),
                        )
                        for R in c_survivors(rqh, mc):
                            rq = R - rqh * RQH
                            ps = psMM.tile([P, FB], f32, tag="mm")
                            for k in range(0, nkn, 2):
                                nc.tensor.matmul(
                                    ps[:],
                                    lh[:, k:k + 2, rq * P:(rq + 1) * P],
                                    rt_[:, k:k + 2, :],
                                    start=(k == 0),
                                    stop=(k == nkn - 2),
                                    perf_mode=DR,
                                )
                            rg = R * P
                            ev32 = evict.tile([P, FB], f32, tag="evC32", bufs=3)
                            nc.vector.tensor_copy(ev32[:], ps[:])
                            nc.sync.dma_start(
                                bh2[rg:rg + P, mc * FB:(mc + 1) * FB], ev32[:]
                            )
                            ev = evict.tile([P, FB], f16, tag="evC", bufs=3)
                            nc.vector.tensor_copy(ev[:], ps[:])
                            pst = psTR.tile([P, FB], f16, tag="trp")
                            for q in range(TPB):
                                nc.tensor.transpose(
                                    pst[:, q * P:(q + 1) * P],
                                    ev[:, q * P:(q + 1) * P],
                                    identh[:],
                                )
                            tT_ = evict.tile([P, FB // P, P], f16, tag="tTC",
                                             bufs=3)
                            for q in range(TPB):
                                nc.vector.tensor_copy(
                                    tT_[:, q, :], pst[:, q * P:(q + 1) * P]
                                )
                            nc.sync.dma_start(
                                bh2T[
                                    mc * FB:(mc + 1) * FB, rg:rg + P
                                ].rearrange("(j p) q -> p j q", p=P),
                                tT_[:],
                            )
                            # mirror stores covering the skipped blocks
                            if mc < nfb // 2:
                                for q in range(TPB):
                                    c = mc * TPB + q
                                    if c <= TPB * (R // TPB) + TPB - 1:
                                        continue
                                    nc.sync.dma_start(
                                        bh2T[R * P:(R + 1) * P,
                                             c * P:(c + 1) * P],
                                        ev[:, q * P:(q + 1) * P],
                                    )
                                    p32 = psTR.tile([P, P], f32, tag="trp")
                                    nc.tensor.transpose(
                                        p32[:], ev32[:, q * P:(q + 1) * P],
                                        ident32[:],
                                    )
                                    mir = evict.tile([P, P], f32, tag="mirC",
                                                     bufs=3)
                                    nc.vector.tensor_copy(mir[:], p32[:])
                                    nc.sync.dma_start(
                                        bh2[c * P:(c + 1) * P,
                                            R * P:(R + 1) * P],
                                        mir[:],
                                    )
                        if pend is not None:
                            hop_stage2(pend)
                            pend = None
                        if n_hop_emitted < nrt:
                            pend = hop_stage1(pe0, 0, n_hop_emitted)
                            n_hop_emitted += 1
                # leftover hop0 row-tiles (pipelined pairs)
                while n_hop_emitted < nrt:
                    st = hop_stage1(pe0, 0, n_hop_emitted)
                    n_hop_emitted += 1
                    if pend is not None:
                        hop_stage2(pend)
                    pend = st
                if pend is not None:
                    hop_stage2(pend)

            # ------- Phase D: bh3 = bh2 @ b0 (f16 x fp8, exact f32) -------
            # + hop1 (this rh) and hop2 (previous rh) row-tiles interleaved
            RH = min(512, r)
            nrh = r // RH
            nrq = RH // P
            with (
                tc.tile_pool(name="pd_lhs", bufs=1) as pdl,
                tc.tile_pool(name="pd_rhs", bufs=2) as pdr,
                tc.tile_pool(name="evDp", bufs=2) as evict,
                tc.tile_pool(name="peD", bufs=2) as peD,
            ):
                TPB = FB // P
                for rh in range(nrh):
                    lh = pdl.tile([P, nkn, RH], f16, tag="lhD")
                    nc.sync.dma_start(
                        lh[:],
                        bh2T[:, rh * RH:(rh + 1) * RH].rearrange(
                            "(k p) q -> p k q", p=P
                        ),
                    )
                    for mc in range(nfb):
                        d_surv = [
                            rh * nrq + rq for rq in range(nrq)
                            if not (mc < nfb // 2
                                    and TPB * mc + TPB - 1 < rh * nrq + rq)
                        ]
                        if not d_surv:
                            continue
                        rt_ = pdr.tile([P, nkn, FB], fp8, tag="rhsD")
                        nc.sync.dma_start(
                            rt_[:],
                            b0q[:, mc * FB:(mc + 1) * FB].rearrange(
                                "(k p) q -> p k q", p=P
                            ),
                        )
                        for R in d_surv:
                            rq = R - rh * nrq
                            ps = psMM.tile([P, FB], f32, tag="mm")
                            for k in range(nkn):
                                nc.tensor.matmul(
                                    ps[:],
                                    lh[:, k, rq * P:(rq + 1) * P],
                                    rt_[:, k, :],
                                    start=(k == 0),
                                    stop=(k == nkn - 1),
                                )
                            ev = evict.tile([P, FB], f32, tag="evD", bufs=4)
                            nc.vector.tensor_copy(ev[:], ps[:])
                            rg = R * P
                            nc.sync.dma_start(
                                bh3[rg:rg + P, mc * FB:(mc + 1) * FB], ev[:]
                            )
                            if mc < nfb // 2:
                                for q in range(TPB):
                                    c = mc * TPB + q
                                    if c <= TPB * (R // TPB) + TPB - 1:
                                        continue
                                    p32 = psTR.tile([P, P], f32, tag="trp")
                                    nc.tensor.transpose(
                                        p32[:], ev[:, q * P:(q + 1) * P],
                                        ident32[:],
                                    )
                                    mir = evict.tile([P, P], f32, tag="mirD",
                                                     bufs=3)
                                    nc.vector.tensor_copy(mir[:], p32[:])
                                    nc.sync.dma_start(
                                        bh3[c * P:(c + 1) * P,
                                            R * P:(R + 1) * P],
                                        mir[:],
                                    )
                    iters = [(1, rh * nrq + j) for j in range(nrq)]
                    if rh > 0:
                        iters += [(2, (rh - 1) * nrq + j) for j in range(nrq)]
                    pend = None
                    for i, rt in iters:
                        st = hop_stage1(peD, i, rt)
                        if pend is not None:
                            hop_stage2(pend)
                        pend = st
                    if pend is not None:
                        hop_stage2(pend)
                # last rh block of hop2
                pend = None
                for j in range(nrq):
                    st = hop_stage1(peD, 2, (nrh - 1) * nrq + j)
                    if pend is not None:
                        hop_stage2(pend)
                    pend = st
                if pend is not None:
                    hop_stage2(pend)

            # ---------------- Phase F: fuse + residual ----------------
            FR = min(512, r)
            nrf = r // FR
            with (
                tc.tile_pool(name="pf", bufs=1) as pf,
                tc.tile_pool(name="evFp", bufs=2) as evict,
            ):
                xp_sb = pf.tile([P, nic, r], f16, tag="xp_sb")
                fuseT_sb = pf.tile([P, nhc, ic], f16, tag="fuseT_sb")
                nc.sync.dma_start(
                    fuseT_sb[:], fuseT[:, :].rearrange("(k p) o -> p k o", p=P)
                )
                resT_sb = pf.tile([P, 2 * nic, cout], f16, tag="resT_sb")
                nc.sync.dma_start(
                    resT_sb[:], resT[:, :].rearrange("(k p) o -> p k o", p=P)
                )
                xres_sb = pf.tile([P, nxc, r], f16, tag="xres_sb")
                nc.sync.dma_start(
                    xres_sb[:], xb[0:ic, 0:r].rearrange("(k p) q -> p k q", p=P)
                )
                for rf in range(nrf):
                    for o in range(nic):
                        ps = psMM.tile([P, FR], f32, tag="mm")
                        for k in range(nhc):
                            nc.tensor.matmul(
                                ps[:],
                                fuseT_sb[:, k, o * P:(o + 1) * P],
                                hops_sb[:, k, rf * FR:(rf + 1) * FR],
                                start=(k == 0),
                                stop=(k == nhc - 1),
                            )
                        nc.vector.tensor_scalar_add(
                            xp_sb[:, o, rf * FR:(rf + 1) * FR],
                            ps[:],
                            bias_sb[:, C_FUSE + o:C_FUSE + o + 1],
                        )

                for o in range(ncout):
                    for rf in range(nrf):
                        ps = psMM.tile([P, FR], f32, tag="mm")
                        for k in range(nxc):
                            nc.tensor.matmul(
                                ps[:],
                                resT_sb[:, k, o * P:(o + 1) * P],
                                xres_sb[:, k, rf * FR:(rf + 1) * FR],
                                start=(k == 0),
                                stop=False,
                            )
                        for k in range(nic):
                            nc.tensor.matmul(
                                ps[:],
                                resT_sb[:, nxc + k, o * P:(o + 1) * P],
                                xp_sb[:, k, rf * FR:(rf + 1) * FR],
                                start=False,
                                stop=(k == nic - 1),
                            )
                        ev = evict.tile([P, FR], f32, tag="evF")
                        nc.vector.tensor_scalar_add(
                            ev[:], ps[:], bias_sb[:, C_RES + o:C_RES + o + 1]
                        )
                        nc.sync.dma_start(
                            out[o * P:(o + 1) * P, rf * FR:(rf + 1) * FR], ev[:]
                        )

    nc.compile()
    return nc


def _host_prep(inputs, cin, ic, n, r, hop, eps):
    """Fold BN into weights; build per-core input maps."""

    def f32(a):
        return np.ascontiguousarray(np.asarray(a, dtype=np.float32))

    x = f32(inputs["x"])
    B = x.shape[0]
    xf = x.reshape(B, cin, n)

    s4 = float(ic) ** -0.25
    inv1 = 1.0 / np.sqrt(f32(inputs["bn1_v"]) + eps) * f32(inputs["bn1_g"])
    w1_eff = (inv1[:, None] * f32(inputs["w1_w"])) * s4
    b1_eff = (f32(inputs["w1_b"]) * inv1 + f32(inputs["bn1_b"])
              - f32(inputs["bn1_m"]) * inv1) * s4

    invf = 1.0 / np.sqrt(f32(inputs["bnf_v"]) + eps) * f32(inputs["bnf_g"])
    fuse_eff = invf[:, None] * f32(inputs["fuse_w"])
    fuse_b_eff = (f32(inputs["fuse_b"]) * invf + f32(inputs["bnf_b"])
                  - f32(inputs["bnf_m"]) * invf)

    invr = 1.0 / np.sqrt(f32(inputs["bnr_v"]) + eps) * f32(inputs["bnr_g"])
    res_eff = invr[:, None] * f32(inputs["res_w"])
    res_b_eff = (f32(inputs["res_b"]) * invr + f32(inputs["bnr_b"])
                 - f32(inputs["bnr_m"]) * invr)

    delta = float(np.asarray(inputs["delta"]).reshape(-1)[0])
    if delta <= 0.0:
        thr = -3.0e38
    elif delta >= 1.0:
        thr = 3.0e38
    else:
        thr = float(np.log(delta / (1.0 - delta)))

    nic = ic // P
    ncout = cin // P
    nbias = nic + nic + hop * nic + nic + ncout
    bias_pack = np.zeros((P, nbias), np.float32)
    col = 0
    for oc in range(nic):
        bias_pack[:, col] = b1_eff[oc * P:(oc + 1) * P]
        col += 1
    node_b = f32(inputs["node_b"])
    for oc in range(nic):
        bias_pack[:, col] = node_b[oc * P:(oc + 1) * P]
        col += 1
    hop_b = f32(inputs["hop_b"])
    for i in range(hop):
        for oc in range(nic):
            bias_pack[:, col] = hop_b[i, oc * P:(oc + 1) * P]
            col += 1
    for oc in range(nic):
        bias_pack[:, col] = fuse_b_eff[oc * P:(oc + 1) * P]
        col += 1
    for oc in range(ncout):
        bias_pack[:, col] = res_b_eff[oc * P:(oc + 1) * P]
        col += 1

    f16 = np.float16
    shared = {
        "w1T": np.ascontiguousarray(w1_eff.T.astype(f16)),
        "nodeT": np.ascontiguousarray(f32(inputs["node_w"]).T.astype(f16)),
        "hopT": np.ascontiguousarray(
            f32(inputs["hop_w"]).transpose(0, 2, 1).astype(f16)
        ),
        "fuseT": np.ascontiguousarray(fuse_eff.T.astype(f16)),
        "resT": np.ascontiguousarray(res_eff.T.astype(f16)),
        "biases": bias_pack,
    }

    n_cores = (B * n) // r
    halves = n // r
    in_maps = []
    for c in range(n_cores):
        b, h = c // halves, c % halves
        perm = (np.arange(n) + h * r) % n
        m = dict(shared)
        m["xb"] = np.ascontiguousarray(xf[b][:, perm].astype(f16))
        in_maps.append(m)
    return in_maps, thr


_BUILD_CACHE = {}


def kernel(**inputs):
    from concourse import bass_utils

    cin, ic, hop, eps = 512, 256, 3, 1e-5
    x = np.asarray(inputs["x"])
    B, _, H, W = x.shape
    n = H * W
    n_cores = 8
    r = (B * n) // n_cores
    halves = n // r

    in_maps, thr = _host_prep(inputs, cin, ic, n, r, hop, eps)

    key = (cin, ic, n, r, hop, thr)
    if key not in _BUILD_CACHE:
        _BUILD_CACHE[key] = _build(cin, ic, n, r, hop, thr)
    nc = _BUILD_CACHE[key]

    res = bass_utils.run_bass_kernel_spmd(nc, in_maps, core_ids=list(range(n_cores)))

    out = np.empty((B, cin, n), np.float32)
    for c in range(n_cores):
        b, h = c // halves, c % halves
        out[b][:, h * r:(h + 1) * r] = res.results[c]["out"]
    return out.reshape(B, cin, H, W).astype(x.dtype)


# revision 51
# speedup vs baseline: 1.2789x; 1.0755x over previous
"""Trainium2 Bass kernel for nn_HA_unit (gnn_message_passing).

Math (per batch b, N = H*W spatial positions):
  wfeat = BN1(w1 @ x)                       [IC, N]   (BN folded on host)
  iw    = wfeat^T wfeat * IC^-0.5           [N, N]    symmetric
  nodes = node_w @ x + node_b               [N, IC]
  b0    = (sigmoid(iw) >= delta)            [N, N]    binary, symmetric
  bh_k  = b0^k  (k = 1, 2, 3)               exact integer counts
  hop_k = hopw_k @ (softmax(bh_k o iw) @ nodes)^T + hopb_k
  xp    = BNf(fuse_w @ concat(hops))
  out   = BNr(res_w @ concat(x[:IC], xp))

Sharding: 8 cores = 4 batches x 2 halves of N. Core (b, h) receives x[b]
with spatial positions rolled by h*N/2 so that its rows are always 0..N/2-1
(identical SPMD program, data-only difference). Each core computes the full
symmetric b0 locally (no collectives); the b0^2 / b0^3 matmuls are sharded
by output rows.

Performance structure:
  - all dense matmuls on f16 operands (1 cyc/row); fp8 DoubleRow
    (0.5 cyc/row) for b0 @ b0.  b0 is exact in fp8, bh2 (< 2048) exact
    in f16, bh3 exact in f32 PSUM.
  - single 4-deep PSUM pool for every accumulation group keeps TensorE
    several groups ahead of the evictions (HAM stays un-throttled).
  - softmax hop row-tiles run split-stage (stage1 = DMA+logits+exp on
    DVE/POOL/ACT, stage2 = transposes + matmuls on PE) with a one-block
    skew, interleaved into the big matmul phases: hop0 inside C, hop1
    and hop2 inside D.  Only fuse+residual remain as a tail.
  - nodes / hops stay SBUF-resident; evictions ride DVE, not ACT.
"""

import sys

sys.path.insert(0, "/opt/trn_rl_repo")

import numpy as np

P = 128


def _build(cin, ic, n, r, hop, thr):
    from concourse import bass, tile, bacc
    import concourse.mybir as mybir
    from concourse.masks import make_identity

    f32 = mybir.dt.float32
    f16 = mybir.dt.float16
    fp8 = mybir.dt.float8e4
    AF = mybir.ActivationFunctionType
    ALU = mybir.AluOpType
    AX = mybir.AxisListType
    DR = mybir.MatmulPerfMode.DoubleRow

    ncin = cin // P          # K-chunks over input channels
    nic = ic // P            # chunks over inter channels
    nkn = n // P             # K-chunks over N
    nrt = r // P             # our row tiles
    FB = min(512, n)         # free-dim blocking
    nfb = n // FB
    hc = hop * ic
    nhc = hc // P
    cout = cin
    ncout = cout // P
    nxc = ic // P            # x residual slice chunks (x[:ic])

    # bias_pack columns: [b1(nic) | nodeb(nic) | hop(hop*nic) | fuse(nic) | res(ncout)]
    C_B1 = 0
    C_NB = C_B1 + nic
    C_HOP = C_NB + nic
    C_FUSE = C_HOP + hop * nic
    C_RES = C_FUSE + nic
    NBIAS = C_RES + ncout

    nc = bacc.Bacc("TRN2", target_bir_lowering=False, debug=False)

    xb = nc.dram_tensor("xb", [cin, n], f16, kind="ExternalInput")
    w1T = nc.dram_tensor("w1T", [cin, ic], f16, kind="ExternalInput")
    nodeT = nc.dram_tensor("nodeT", [cin, ic], f16, kind="ExternalInput")
    hopT = nc.dram_tensor("hopT", [hop, ic, ic], f16, kind="ExternalInput")
    fuseT = nc.dram_tensor("fuseT", [hc, ic], f16, kind="ExternalInput")
    resT = nc.dram_tensor("resT", [2 * ic, cout], f16, kind="ExternalInput")
    biases = nc.dram_tensor("biases", [P, NBIAS], f32, kind="ExternalInput")
    out = nc.dram_tensor("out", [cout, r], f32, kind="ExternalOutput")

    with tile.TileContext(nc) as tc:
        with (
            tc.tile_pool(name="dram", bufs=1, space="DRAM") as dpool,
            tc.tile_pool(name="consts", bufs=1) as consts,
            tc.tile_pool(name="res", bufs=1) as resid,
            tc.tile_pool(name="psMM", bufs=3, space="PSUM") as psMM,
            tc.tile_pool(name="psTR", bufs=3, space="PSUM") as psTR,
        ):
            # b0 split into column halves and bh3 into row blocks so phase
            # entries depend only on the ranges actually written (the DRAM
            # dependency tracker is per-tile).
            b0qL = dpool.tile([n, r], fp8, tag="b0qL")
            b0qR = dpool.tile([n, n - r], fp8, tag="b0qR")
            iwq = dpool.tile([r, n], f32, tag="iwq")
            bh2 = dpool.tile([r, n], f32, tag="bh2")
            bh2T = dpool.tile([n, r], f16, tag="bh2T")
            RH_ = min(512, r)
            bh3t = [
                dpool.tile([RH_, n], f32, tag=f"bh3_{i}", name=f"bh3_{i}")
                for i in range(r // RH_)
            ]

            def b0q_col(mc, FBw):
                half = r // FBw
                if mc < half:
                    return b0qL[:, mc * FBw:(mc + 1) * FBw]
                return b0qR[:, (mc - half) * FBw:(mc - half + 1) * FBw]

            identh = consts.tile([P, P], f16, tag="identh")
            make_identity(nc, identh[:])
            ident32 = consts.tile([P, P], f32, tag="ident32")
            make_identity(nc, ident32[:])
            bias_sb = consts.tile([P, NBIAS], f32, tag="bias_sb")
            nc.sync.dma_start(bias_sb[:], biases[:])

            # persistent SBUF residents
            nodes_sb = resid.tile([P, nkn, ic], f16, tag="nodes_sb")
            hops_sb = resid.tile([P, hop * nic, r], f16, tag="hops_sb")
            hopT_sb = resid.tile([P, hop, nic, ic], f16, tag="hopT_sb")
            for i in range(hop):
                nc.sync.dma_start(
                    hopT_sb[:, i, :, :],
                    hopT[i, :, :].rearrange("(k p) o -> p k o", p=P),
                )

            # ---- softmax hop row-tile, split in two stages ----
            # stage1: load iw row + bh row, logits, rowmax, exp (+rowsum)
            # stage2: transpose s, t = s @ nodes (scaled 1/Z), hop conv
            def hop_stage1(pool, i, rt, src_rows=None):
                iwt = pool.tile([P, n], f32, tag="E_iw")
                nc.sync.dma_start(iwt[:], iwq[rt * P:(rt + 1) * P, :])
                if i == 0:
                    bht = pool.tile([P, n], fp8, tag="E_bh0")
                    nc.sync.dma_start(
                        bht[:, 0:r], b0qL[rt * P:(rt + 1) * P, :]
                    )
                    nc.sync.dma_start(
                        bht[:, r:n], b0qR[rt * P:(rt + 1) * P, :]
                    )
                    nc.vector.tensor_mul(iwt[:], iwt[:], bht[:])
                else:
                    bht = pool.tile([P, n], f32, tag="E_bh", bufs=2)
                    nc.sync.dma_start(bht[:], src_rows)
                    # split the logits multiply across POOL and DVE to
                    # shorten the stage1 chain
                    nh = n // 2
                    nc.gpsimd.tensor_mul(
                        iwt[:, 0:nh], iwt[:, 0:nh], bht[:, 0:nh]
                    )
                    nc.vector.tensor_mul(
                        iwt[:, nh:n], iwt[:, nh:n], bht[:, nh:n]
                    )
                nmax = pool.tile([P, 1], f32, tag="E_nm")
                nc.vector.tensor_reduce(
                    nmax[:], iwt[:], axis=AX.X, op=ALU.max, negate=True
                )
                zt = pool.tile([P, 1], f32, tag="E_z")
                pt_ = pool.tile([P, n], f16, tag="E_p")
                nc.scalar.activation(
                    pt_[:], iwt[:], AF.Exp, bias=nmax[:], accum_out=zt[:]
                )
                rz = pool.tile([P, 1], f32, tag="E_rz")
                nc.vector.reciprocal(rz[:], zt[:])
                return (pool, i, rt, pt_, rz)

            def hop_stage2(state):
                pool, i, rt, pt_, rz = state
                sT = pool.tile([P, nfb, FB], f16, tag="E_sT", bufs=1)
                for jb in range(nfb):
                    pst = psTR.tile([P, FB], f16, tag="trp")
                    for q in range(FB // P):
                        nc.tensor.transpose(
                            pst[:, q * P:(q + 1) * P],
                            pt_[:, jb * FB + q * P:jb * FB + (q + 1) * P],
                            identh[:],
                        )
                    nc.vector.tensor_copy(sT[:, jb, :], pst[:])
                tps = psMM.tile([P, FB], f32, tag="tps", bufs=2)
                for j in range(nkn):
                    nc.tensor.matmul(
                        tps[:, 0:ic],
                        sT[:, j // (FB // P), (j % (FB // P)) * P:
                           (j % (FB // P)) * P + P],
                        nodes_sb[:, j, :],
                        start=(j == 0),
                        stop=(j == nkn - 1),
                    )
                t_sb = pool.tile([P, ic], f16, tag="E_t")
                nc.vector.tensor_scalar_mul(t_sb[:], tps[:, 0:ic], rz[:])
                tT = pool.tile([P, nic, P], f16, tag="E_tT")
                for q in range(nic):
                    ptq = psTR.tile([P, P], f16, tag="trp")
                    nc.tensor.transpose(
                        ptq[:], t_sb[:, q * P:(q + 1) * P], identh[:]
                    )
                    nc.vector.tensor_copy(tT[:, q, :], ptq[:])
                for o in range(nic):
                    ph = psTR.tile([P, P], f32, tag="trp")
                    for c in range(nic):
                        nc.tensor.matmul(
                            ph[:],
                            hopT_sb[:, i, c, o * P:(o + 1) * P],
                            tT[:, c, :],
                            start=(c == 0),
                            stop=(c == nic - 1),
                        )
                    nc.vector.tensor_scalar_add(
                        hops_sb[:, i * nic + o, rt * P:(rt + 1) * P],
                        ph[:],
                        bias_sb[
                            :, C_HOP + i * nic + o:C_HOP + i * nic + o + 1
                        ],
                    )

            # ---------------- Phase A: wfeat + nodes ----------------
            with (
                tc.tile_pool(name="pa", bufs=1) as pa,
                tc.tile_pool(name="evA", bufs=2) as evict,
            ):
                w1T_sb = pa.tile([P, ncin, ic], f16, tag="w1T_sb")
                nc.sync.dma_start(
                    w1T_sb[:], w1T[:, :].rearrange("(k p) o -> p k o", p=P)
                )
                nodeT_sb = pa.tile([P, ncin, ic], f16, tag="nodeT_sb")
                nc.sync.dma_start(
                    nodeT_sb[:], nodeT[:, :].rearrange("(k p) o -> p k o", p=P)
                )
                # per-k x loads so the first wfeat group starts early
                x_sb = pa.tile([P, ncin, n], f16, tag="x_sb")
                for k in range(ncin):
                    nc.sync.dma_start(
                        x_sb[:, k, :], xb[k * P:(k + 1) * P, :]
                    )
                wf_sb = pa.tile([P, nic, n], f16, tag="wf_sb")
                ndT_sb = pa.tile([P, nic, n], f16, tag="ndT_sb")

                for oc in range(nic):
                    for f in range(nfb):
                        ps = psMM.tile([P, FB], f32, tag="mm")
                        for k in range(ncin):
                            nc.tensor.matmul(
                                ps[:],
                                w1T_sb[:, k, oc * P:(oc + 1) * P],
                                x_sb[:, k, f * FB:(f + 1) * FB],
                                start=(k == 0),
                                stop=(k == ncin - 1),
                            )
                        nc.vector.tensor_scalar_add(
                            wf_sb[:, oc, f * FB:(f + 1) * FB],
                            ps[:],
                            bias_sb[:, C_B1 + oc:C_B1 + oc + 1],
                        )
                for oc in range(nic):
                    for f in range(nfb):
                        ps = psMM.tile([P, FB], f32, tag="mm")
                        for k in range(ncin):
                            nc.tensor.matmul(
                                ps[:],
                                nodeT_sb[:, k, oc * P:(oc + 1) * P],
                                x_sb[:, k, f * FB:(f + 1) * FB],
                                start=(k == 0),
                                stop=(k == ncin - 1),
                            )
                        nc.vector.tensor_scalar_add(
                            ndT_sb[:, oc, f * FB:(f + 1) * FB],
                            ps[:],
                            bias_sb[:, C_NB + oc:C_NB + oc + 1],
                        )
                # nodes_sb[j, :] rows via PE transposes of ndT
                for j in range(nkn):
                    pst = psTR.tile([P, FB], f16, tag="trp")
                    for c in range(nic):
                        nc.tensor.transpose(
                            pst[:, c * P:(c + 1) * P],
                            ndT_sb[:, c, j * P:(j + 1) * P],
                            identh[:],
                        )
                    nc.vector.tensor_copy(nodes_sb[:, j, :], pst[:, 0:ic])

                # ---------------- Phase B: iw + b0 ----------------
                # f-outer so the left half of b0q completes early and Phase
                # C's stationary-operand load can prefetch during B's tail.
                for f in range(nfb):
                    for pc in range(nkn):
                        ps = psMM.tile([P, FB], f32, tag="mm")
                        for k in range(nic):
                            nc.tensor.matmul(
                                ps[:],
                                wf_sb[:, k, pc * P:(pc + 1) * P],
                                wf_sb[:, k, f * FB:(f + 1) * FB],
                                start=(k == 0),
                                stop=(k == nic - 1),
                            )
                        b0t = evict.tile([P, FB], fp8, tag="b0t", bufs=6)
                        nc.vector.tensor_scalar(
                            b0t[:], ps[:], thr, None, op0=ALU.is_ge
                        )
                        nc.sync.dma_start(
                            b0q_col(f, FB)[pc * P:(pc + 1) * P, :], b0t[:]
                        )
                        if pc * P < r:
                            iwt = evict.tile([P, FB], f32, tag="iwt", bufs=4)
                            nc.scalar.activation(iwt[:], ps[:], AF.Copy)
                            nc.sync.dma_start(
                                iwq[pc * P:(pc + 1) * P, f * FB:(f + 1) * FB],
                                iwt[:],
                            )

            # ------- Phase C: bh2 = b0 @ b0 (fp8 DoubleRow, exact) -------
            # + hop0 row-tiles, split-stage, one block of skew
            with (
                tc.tile_pool(name="pc_lhs", bufs=1) as pcl,
                tc.tile_pool(name="pc_rhs", bufs=2) as pcr,
                tc.tile_pool(name="evCp", bufs=2) as evict,
                tc.tile_pool(name="pe0", bufs=2) as pe0,
            ):
                RQH = nrt // 2
                TPB = FB // P
                pend = None
                n_hop_emitted = 0

                def c_survivors(rqh, mc):
                    return [
                        rqh * RQH + rq
                        for rq in range(RQH)
                        if not (mc < nfb // 2
                                and TPB * mc + TPB - 1 < rqh * RQH + rq)
                    ]

                blocks = [(rqh, mc) for rqh in range(2) for mc in range(nfb)
                          if c_survivors(rqh, mc)]
                last_rqh = None
                for rqh, mc in blocks:
                    if rqh != last_rqh:
                        lh = pcl.tile([P, nkn, RQH * P], fp8, tag="lhC")
                        nc.sync.dma_start(
                            lh[:],
                            b0qL[:, rqh * RQH * P:(rqh + 1) * RQH * P].rearrange(
                                "(k p) q -> p k q", p=P
                            ),
                        )
                        last_rqh = rqh
                    if True:
                        rt_ = pcr.tile([P, nkn, FB], fp8, tag="rhsC")
                        nc.sync.dma_start(
                            rt_[:],
                            b0q_col(mc, FB).rearrange("(k p) q -> p k q", p=P),
                        )
                        for R in c_survivors(rqh, mc):
                            rq = R - rqh * RQH
                            ps = psMM.tile([P, FB], f32, tag="mm")
                            for k in range(0, nkn, 2):
                                nc.tensor.matmul(
                                    ps[:],
                                    lh[:, k:k + 2, rq * P:(rq + 1) * P],
                                    rt_[:, k:k + 2, :],
                                    start=(k == 0),
                                    stop=(k == nkn - 2),
                                    perf_mode=DR,
                                )
                            rg = R * P
                            ev32 = evict.tile([P, FB], f32, tag="evC32", bufs=3)
                            nc.vector.tensor_copy(ev32[:], ps[:])
                            nc.sync.dma_start(
                                bh2[rg:rg + P, mc * FB:(mc + 1) * FB], ev32[:]
                            )
                            ev = evict.tile([P, FB], f16, tag="evC", bufs=3)
                            nc.vector.tensor_copy(ev[:], ps[:])
                            pst = psTR.tile([P, FB], f16, tag="trp")
                            for q in range(TPB):
                                nc.tensor.transpose(
                                    pst[:, q * P:(q + 1) * P],
                                    ev[:, q * P:(q + 1) * P],
                                    identh[:],
                                )
                            tT_ = evict.tile([P, FB // P, P], f16, tag="tTC",
                                             bufs=3)
                            for q in range(TPB):
                                nc.vector.tensor_copy(
                                    tT_[:, q, :], pst[:, q * P:(q + 1) * P]
                                )
                            nc.sync.dma_start(
                                bh2T[
                                    mc * FB:(mc + 1) * FB, rg:rg + P
                                ].rearrange("(j p) q -> p j q", p=P),
                                tT_[:],
                            )
                            # mirror stores covering the skipped blocks
                            if mc < nfb // 2:
                                for q in range(TPB):
                                    c = mc * TPB + q
                                    if c <= TPB * (R // TPB) + TPB - 1:
                                        continue
                                    nc.sync.dma_start(
                                        bh2T[R * P:(R + 1) * P,
                                             c * P:(c + 1) * P],
                                        ev[:, q * P:(q + 1) * P],
                                    )
                                    p32 = psTR.tile([P, P], f32, tag="trp")
                                    nc.tensor.transpose(
                                        p32[:], ev32[:, q * P:(q + 1) * P],
                                        ident32[:],
                                    )
                                    mir = evict.tile([P, P], f32, tag="mirC",
                                                     bufs=3)
                                    nc.vector.tensor_copy(mir[:], p32[:])
                                    nc.sync.dma_start(
                                        bh2[c * P:(c + 1) * P,
                                            R * P:(R + 1) * P],
                                        mir[:],
                                    )
                        if pend is not None:
                            hop_stage2(pend)
                            pend = None
                        if n_hop_emitted < nrt:
                            pend = hop_stage1(pe0, 0, n_hop_emitted)
                            n_hop_emitted += 1
                # leftover hop0 row-tiles (pipelined pairs)
                while n_hop_emitted < nrt:
                    st = hop_stage1(pe0, 0, n_hop_emitted)
                    n_hop_emitted += 1
                    if pend is not None:
                        hop_stage2(pend)
                    pend = st
                if pend is not None:
                    hop_stage2(pend)

            # ------- Phase D: bh3 = bh2 @ b0 (f16 x fp8, exact f32) -------
            # + hop1 (this rh) and hop2 (previous rh) row-tiles interleaved
            RH = min(512, r)
            nrh = r // RH
            nrq = RH // P
            with (
                tc.tile_pool(name="pd_lhs", bufs=1) as pdl,
                tc.tile_pool(name="pd_rhs", bufs=2) as pdr,
                tc.tile_pool(name="evDp", bufs=2) as evict,
                tc.tile_pool(name="peD", bufs=2) as peD,
            ):
                TPB = FB // P
                for rh in range(nrh):
                    lh = pdl.tile([P, nkn, RH], f16, tag="lhD")
                    nc.sync.dma_start(
                        lh[:],
                        bh2T[:, rh * RH:(rh + 1) * RH].rearrange(
                            "(k p) q -> p k q", p=P
                        ),
                    )
                    for mc in range(nfb):
                        d_surv = [
                            rh * nrq + rq for rq in range(nrq)
                            if not (mc < nfb // 2
                                    and TPB * mc + TPB - 1 < rh * nrq + rq)
                        ]
                        if not d_surv:
                            continue
                        rt_ = pdr.tile([P, nkn, FB], fp8, tag="rhsD")
                        nc.sync.dma_start(
                            rt_[:],
                            b0q_col(mc, FB).rearrange("(k p) q -> p k q", p=P),
                        )
                        for R in d_surv:
                            rq = R - rh * nrq
                            ps = psMM.tile([P, FB], f32, tag="mm")
                            for k in range(nkn):
                                nc.tensor.matmul(
                                    ps[:],
                                    lh[:, k, rq * P:(rq + 1) * P],
                                    rt_[:, k, :],
                                    start=(k == 0),
                                    stop=(k == nkn - 1),
                                )
                            ev = evict.tile([P, FB], f32, tag="evD", bufs=4)
                            nc.vector.tensor_copy(ev[:], ps[:])
                            ro = (R % nrq) * P
                            nc.sync.dma_start(
                                bh3t[R // nrq][ro:ro + P,
                                               mc * FB:(mc + 1) * FB],
                                ev[:],
                            )
                            if mc < nfb // 2:
                                for q in range(TPB):
                                    c = mc * TPB + q
                                    if c <= TPB * (R // TPB) + TPB - 1:
                                        continue
                                    p32 = psTR.tile([P, P], f32, tag="trp")
                                    nc.tensor.transpose(
                                        p32[:], ev[:, q * P:(q + 1) * P],
                                        ident32[:],
                                    )
                                    mir = evict.tile([P, P], f32, tag="mirD",
                                                     bufs=3)
                                    nc.vector.tensor_copy(mir[:], p32[:])
                                    co = (c % nrq) * P
                                    nc.sync.dma_start(
                                        bh3t[c // nrq][co:co + P,
                                                       R * P:(R + 1) * P],
                                        mir[:],
                                    )
                    def bh_rows(i, rt):
                        if i == 1:
                            return bh2[rt * P:(rt + 1) * P, :]
                        ro = (rt % nrq) * P
                        return bh3t[rt // nrq][ro:ro + P, :]

                    iters = [(1, rh * nrq + j) for j in range(nrq)]
                    if rh > 0:
                        iters += [(2, (rh - 1) * nrq + j) for j in range(nrq)]
                    pend = None
                    for i, rt in iters:
                        st = hop_stage1(peD, i, rt, bh_rows(i, rt))
                        if pend is not None:
                            hop_stage2(pend)
                        pend = st
                    if pend is not None:
                        hop_stage2(pend)
                # last rh block of hop2
                pend = None
                for j in range(nrq):
                    rt = (nrh - 1) * nrq + j
                    ro = (rt % nrq) * P
                    st = hop_stage1(peD, 2, rt, bh3t[rt // nrq][ro:ro + P, :])
                    if pend is not None:
                        hop_stage2(pend)
                    pend = st
                if pend is not None:
                    hop_stage2(pend)

            # ---------------- Phase F: fuse + residual ----------------
            FR = min(512, r)
            nrf = r // FR
            with (
                tc.tile_pool(name="pf", bufs=1) as pf,
                tc.tile_pool(name="evFp", bufs=2) as evict,
            ):
                xp_sb = pf.tile([P, nic, r], f16, tag="xp_sb")
                fuseT_sb = pf.tile([P, nhc, ic], f16, tag="fuseT_sb")
                nc.sync.dma_start(
                    fuseT_sb[:], fuseT[:, :].rearrange("(k p) o -> p k o", p=P)
                )
                resT_sb = pf.tile([P, 2 * nic, cout], f16, tag="resT_sb")
                nc.sync.dma_start(
                    resT_sb[:], resT[:, :].rearrange("(k p) o -> p k o", p=P)
                )
                xres_sb = pf.tile([P, nxc, r], f16, tag="xres_sb")
                nc.sync.dma_start(
                    xres_sb[:], xb[0:ic, 0:r].rearrange("(k p) q -> p k q", p=P)
                )
                for rf in range(nrf):
                    for o in range(nic):
                        ps = psMM.tile([P, FR], f32, tag="mm")
                        for k in range(nhc):
                            nc.tensor.matmul(
                                ps[:],
                                fuseT_sb[:, k, o * P:(o + 1) * P],
                                hops_sb[:, k, rf * FR:(rf + 1) * FR],
                                start=(k == 0),
                                stop=(k == nhc - 1),
                            )
                        nc.vector.tensor_scalar_add(
                            xp_sb[:, o, rf * FR:(rf + 1) * FR],
                            ps[:],
                            bias_sb[:, C_FUSE + o:C_FUSE + o + 1],
                        )

                for o in range(ncout):
                    for rf in range(nrf):
                        ps = psMM.tile([P, FR], f32, tag="mm")
                        for k in range(nxc):
                            nc.tensor.matmul(
                                ps[:],
                                resT_sb[:, k, o * P:(o + 1) * P],
                                xres_sb[:, k, rf * FR:(rf + 1) * FR],
                                start=(k == 0),
                                stop=False,
                            )
                        for k in range(nic):
                            nc.tensor.matmul(
                                ps[:],
                                resT_sb[:, nxc + k, o * P:(o + 1) * P],
                                xp_sb[:, k, rf * FR:(rf + 1) * FR],
                                start=False,
                                stop=(k == nic - 1),
                            )
                        ev = evict.tile([P, FR], f32, tag="evF")
                        nc.vector.tensor_scalar_add(
                            ev[:], ps[:], bias_sb[:, C_RES + o:C_RES + o + 1]
                        )
                        nc.sync.dma_start(
                            out[o * P:(o + 1) * P, rf * FR:(rf + 1) * FR], ev[:]
                        )

    nc.compile()
    return nc


def _host_prep(inputs, cin, ic, n, r, hop, eps):
    """Fold BN into weights; build per-core input maps."""

    def f32(a):
        return np.ascontiguousarray(np.asarray(a, dtype=np.float32))

    x = f32(inputs["x"])
    B = x.shape[0]
    xf = x.reshape(B, cin, n)

    s4 = float(ic) ** -0.25
    inv1 = 1.0 / np.sqrt(f32(inputs["bn1_v"]) + eps) * f32(inputs["bn1_g"])
    w1_eff = (inv1[:, None] * f32(inputs["w1_w"])) * s4
    b1_eff = (f32(inputs["w1_b"]) * inv1 + f32(inputs["bn1_b"])
              - f32(inputs["bn1_m"]) * inv1) * s4

    invf = 1.0 / np.sqrt(f32(inputs["bnf_v"]) + eps) * f32(inputs["bnf_g"])
    fuse_eff = invf[:, None] * f32(inputs["fuse_w"])
    fuse_b_eff = (f32(inputs["fuse_b"]) * invf + f32(inputs["bnf_b"])
                  - f32(inputs["bnf_m"]) * invf)

    invr = 1.0 / np.sqrt(f32(inputs["bnr_v"]) + eps) * f32(inputs["bnr_g"])
    res_eff = invr[:, None] * f32(inputs["res_w"])
    res_b_eff = (f32(inputs["res_b"]) * invr + f32(inputs["bnr_b"])
                 - f32(inputs["bnr_m"]) * invr)

    delta = float(np.asarray(inputs["delta"]).reshape(-1)[0])
    if delta <= 0.0:
        thr = -3.0e38
    elif delta >= 1.0:
        thr = 3.0e38
    else:
        thr = float(np.log(delta / (1.0 - delta)))

    nic = ic // P
    ncout = cin // P
    nbias = nic + nic + hop * nic + nic + ncout
    bias_pack = np.zeros((P, nbias), np.float32)
    col = 0
    for oc in range(nic):
        bias_pack[:, col] = b1_eff[oc * P:(oc + 1) * P]
        col += 1
    node_b = f32(inputs["node_b"])
    for oc in range(nic):
        bias_pack[:, col] = node_b[oc * P:(oc + 1) * P]
        col += 1
    hop_b = f32(inputs["hop_b"])
    for i in range(hop):
        for oc in range(nic):
            bias_pack[:, col] = hop_b[i, oc * P:(oc + 1) * P]
            col += 1
    for oc in range(nic):
        bias_pack[:, col] = fuse_b_eff[oc * P:(oc + 1) * P]
        col += 1
    for oc in range(ncout):
        bias_pack[:, col] = res_b_eff[oc * P:(oc + 1) * P]
        col += 1

    f16 = np.float16
    shared = {
        "w1T": np.ascontiguousarray(w1_eff.T.astype(f16)),
        "nodeT": np.ascontiguousarray(f32(inputs["node_w"]).T.astype(f16)),
        "hopT": np.ascontiguousarray(
            f32(inputs["hop_w"]).transpose(0, 2, 1).astype(f16)
        ),
        "fuseT": np.ascontiguousarray(fuse_eff.T.astype(f16)),
        "resT": np.ascontiguousarray(res_eff.T.astype(f16)),
        "biases": bias_pack,
    }

    n_cores = (B * n) // r
    halves = n // r
    in_maps = []
    for c in range(n_cores):
        b, h = c // halves, c % halves
        perm = (np.arange(n) + h * r) % n
        m = dict(shared)
        m["xb"] = np.ascontiguousarray(xf[b][:, perm].astype(f16))
        in_maps.append(m)
    return in_maps, thr


_BUILD_CACHE = {}


def kernel(**inputs):
    from concourse import bass_utils

    cin, ic, hop, eps = 512, 256, 3, 1e-5
    x = np.asarray(inputs["x"])
    B, _, H, W = x.shape
    n = H * W
    n_cores = 8
    r = (B * n) // n_cores
    halves = n // r

    in_maps, thr = _host_prep(inputs, cin, ic, n, r, hop, eps)

    key = (cin, ic, n, r, hop, thr)
    if key not in _BUILD_CACHE:
        _BUILD_CACHE[key] = _build(cin, ic, n, r, hop, thr)
    nc = _BUILD_CACHE[key]

    res = bass_utils.run_bass_kernel_spmd(nc, in_maps, core_ids=list(range(n_cores)))

    out = np.empty((B, cin, n), np.float32)
    for c in range(n_cores):
        b, h = c // halves, c % halves
        out[b][:, h * r:(h + 1) * r] = res.results[c]["out"]
    return out.reshape(B, cin, H, W).astype(x.dtype)
